# revision 1
# baseline (speedup 1.0000x reference)
"""Trainium2 Bass kernel for nn_AgnisV5 (B=4, T=256, V=50257, D=768, H=3072).

Strategy
--------
The reference is a 256-step sequential recurrence over h (LayerNorm'd each
step) plus a big lm_head projection that does not feed back. The recurrence
map is contractive (Jacobian norm ~0.65), so instead of stepping 256 times
with tiny (M=4) matmuls, we solve the whole sequence by 13 batched Picard
sweeps: H <- StepAll(shift(H)), each sweep a full-width (M=128/core) pass
over all timesteps. Validated numerically: error ~0.65^k; bf16 sweeps floor
at ~2.8e-3 of logits scale; measured end-to-end max-err ~7e-3 of scale.

Sharding: time-sharded across 8 cores (128 rows = 32 timesteps x batch 4 per
core), weights replicated in bf16 SBUF-resident form. The only cross-core
traffic is a tiny per-sweep boundary halo (lag-2, fully overlapped AllGather)
plus one final AllGather of H for the vocab-sharded fp32r lm_head.

Math simplifications (validated vs reference in fp64):
  - x2 relaxation collapses: x2 = (1-c) * target (+O(c)=1e-3 terms that
    vanish under l2-normalize), so core_blended = l2n(target).
  - gelu(x) = 0.5*x*(1+erf(x/sqrt(2))): erf on ScalarE (single ACT table set
    with sigmoid), 0.5 folded into the next weight matrix on the host.
  - temporal_feat = h_prev @ (R @ Wt), folded on host in fp64.
  - rsqrt for l2n/LN via DVE Newton iterations (no ACT table switch).
"""
import sys, os
sys.path.insert(0, '/opt/trn_rl_repo')
import numpy as np
import ml_dtypes

import concourse.bass as bass
import concourse.bacc as bacc
import concourse.mybir as mybir
import concourse.tile as tile
from concourse.bass_utils import run_bass_kernel_spmd


def _ensure_ntff_hook():
    """The agent image's antenv lacks axon_hooks, which silently disables
    NTFF profiling (exec_time_ns). Shim the module and register the
    ctypes-based hook from trn_agent_boot if available."""
    import types
    if "antenv.axon_hooks" in sys.modules:
        return
    try:
        import antenv
        m = types.ModuleType("antenv.axon_hooks")
        _h = [None]
        m.set_axon_ntff_profile_hook = lambda h: _h.__setitem__(0, h)
        m.get_axon_ntff_profile_hook = lambda: _h[0]
        sys.modules["antenv.axon_hooks"] = m
        antenv.axon_hooks = m
        from trn_agent_boot.trn_boot import _ntff_profile_via_ctypes
        hook = _ntff_profile_via_ctypes("/opt/axon/libaxon_pjrt.so")
        if hook is not None:
            m.set_axon_ntff_profile_hook(hook)
    except Exception:
        pass


_ensure_ntff_hook()

F32 = mybir.dt.float32
F32R = mybir.dt.float32r
BF16 = mybir.dt.bfloat16
AF = mybir.ActivationFunctionType
ALU = mybir.AluOpType

N_CORES = 8
B, T, V, D, H = 4, 256, 50257, 768, 3072
ROWS = 128                 # rows per core = 32 timesteps x 4 batch
KC_D = D // 128            # 6 chunks of the d dimension
KC_H = H // 128            # 24 chunks of the hidden dimension
VPAD = 6400                # per-core vocab shard cols, padded to 50*128
VSHARD = 6283              # ceil(V / 8); host pads vocab to 8*VSHARD = 50264
N_ITERS = 13
ALPHA = 0.4
INV_SQRT2 = 0.7071067811865476

LAST_RESULT = None         # BassKernelResults of the most recent run (for test.py)
TILE_NAMES = {}            # debug: logical name -> sim tensor name

_BUILD_CACHE = {}


def _t_layout(w):
    """[K, M] row-major -> [128, K/128, M] T-layout for stationary lhsT tiles."""
    K, M = w.shape
    assert K % 128 == 0
    return np.ascontiguousarray(w.reshape(K // 128, 128, M).transpose(1, 0, 2))


def _nr_rsqrt(nc, pool, s_ap, n_free, name, iters=3):
    """rsqrt(s) on DVE: bit-trick seed + Newton iterations. s_ap: [1, n] f32."""
    bits = pool.tile([1, n_free], mybir.dt.int32, tag=f"{name}_bits")
    nc.vector.tensor_scalar(bits[:], s_ap.bitcast(mybir.dt.int32), 1, None,
                            ALU.logical_shift_right)
    nc.vector.tensor_scalar(bits[:], bits[:], -1, 0x5f3759df, ALU.mult, ALU.add)
    y = pool.tile([1, n_free], F32, tag=f"{name}_y")
    nc.vector.tensor_copy(y[:], bits[:].bitcast(F32))
    half = pool.tile([1, n_free], F32, tag=f"{name}_half")
    nc.vector.tensor_scalar(half[:], s_ap, 0.5, None, ALU.mult)
    yy = pool.tile([1, n_free], F32, tag=f"{name}_yy")
    e = pool.tile([1, n_free], F32, tag=f"{name}_e")
    for _ in range(iters):
        nc.vector.tensor_tensor(yy[:], y[:], y[:], ALU.mult)
        nc.vector.tensor_tensor(e[:], yy[:], half[:], ALU.mult)
        nc.vector.tensor_scalar(e[:], e[:], -1.0, 1.5, ALU.mult, ALU.add)
        nc.vector.tensor_tensor(y[:], y[:], e[:], ALU.mult)
    return y


def build(n_iters=N_ITERS):
    nc = bacc.Bacc("TRN2", target_bir_lowering=False, debug=False,
                   num_devices=N_CORES)

    # ---- DRAM parameters (per-core data via in_maps) ----
    embT_ext = nc.declare_dram_parameter("embT", [128, KC_D, ROWS], F32, isOutput=False)
    sel_ext = nc.declare_dram_parameter("sel", [128, 8], F32, isOutput=False)
    wb_ext = {}
    for name, (wk, wm) in dict(Wgt=(D, D), V0=(D, H), V1=(H, D), W1=(D, D),
                               W2=(D, D), W2Wg=(D, D), RWt=(D, D), R=(D, D)).items():
        wb_ext[name] = nc.declare_dram_parameter(f"wb_{name}", [128, wk // 128, wm],
                                                 BF16, isOutput=False)
    wl_ext = nc.declare_dram_parameter("wl", [VPAD // 128, 128, KC_D, 128], F32, isOutput=False)
    out_ext = nc.declare_dram_parameter("out", [VPAD, T * B], F32, isOutput=True)
    warm_ext = nc.declare_dram_parameter("warm", [128, 4], F32, isOutput=True)

    # ---- internal DRAM for collectives ----
    halo_in = [nc.dram_tensor(f"halo_in_{k}", [128, KC_D * 4], F32)
               for k in range(n_iters)]
    halo_out = [nc.dram_tensor(f"halo_out_{k}", [N_CORES * 128, KC_D * 4], F32,
                               addr_space="Shared") for k in range(n_iters)]
    ccw_in = nc.dram_tensor("ccw_in", [1, 32], F32)
    ccw_out = nc.dram_tensor("ccw_out", [N_CORES, 32], F32, addr_space="Shared")
    hfin_in = nc.dram_tensor("hfin_in", [128, D], F32)
    hfin_out = nc.dram_tensor("hfin_out", [N_CORES * 128, D], F32,
                              addr_space="Shared")

    rg = [list(range(N_CORES))]

    with tile.TileContext(nc) as tc:
        with (
            tc.tile_pool(name="wpool", bufs=1) as wpool,
            tc.tile_pool(name="cpool", bufs=1) as cpool,      # constants / persistents
            tc.tile_pool(name="apool", bufs=1) as apool,      # per-iteration activations
            tc.tile_pool(name="npool", bufs=1) as npool,      # norm scratch
            tc.tile_pool(name="pps", bufs=4, space="PSUM") as pps,
            tc.tile_pool(name="sps", bufs=2, space="PSUM") as sps,
        ):
            # ---------- load persistent data ----------
            wsb = {}
            for name, ext in wb_ext.items():
                shape = list(ext.shape)
                t_ = wpool.tile(shape, BF16, tag=f"w_{name}")
                nc.sync.dma_start(t_[:], ext[:])
                wsb[name] = t_
            embT = cpool.tile([128, KC_D, ROWS], F32, tag="embT")
            nc.sync.dma_start(embT[:], embT_ext[:])
            sel = cpool.tile([128, 8], F32, tag="sel")
            nc.sync.dma_start(sel[:], sel_ext[:])
            embTbf = cpool.tile([128, KC_D, ROWS], BF16, tag="embTbf")
            nc.vector.tensor_copy(embTbf[:], embT[:])
            # warm up the collective path early (first call pays ENCD init)
            nc.sync.dma_start(ccw_in[:], embT[0:1, 0, 0:32])
            nc.gpsimd.collective_compute(
                "AllGather", ALU.bypass, replica_groups=rg,
                ins=[ccw_in[:]], outs=[ccw_out[:]])

            ones_col_bf = cpool.tile([128, 1], BF16, tag="ones_col_bf")
            nc.vector.memset(ones_col_bf[:], 1.0)
            ones_col_f = cpool.tile([128, 1], F32, tag="ones_col_f")
            nc.vector.memset(ones_col_f[:], 1.0)
            ones_row_f = cpool.tile([1, 128], F32, tag="ones_row_f")
            nc.vector.memset(ones_row_f[:], 1.0)

            # helper: one weight "layer": out chunks [mc] = sum_kc lhsT @ rhs
            def mm_layer(wname, Kc, Mc, rhs_fn, consume, group=4, wtile=None,
                         mode="mgroup"):
                """consume(psum_ap, m0, nchunks) handles [128, nchunks*128] out.

                mode="kouter": single wide psum accumulator, kc outer loop --
                consumes rhs chunks as soon as they are produced (removes the
                inter-layer barrier) and yields one wide consume."""
                w = wtile if wtile is not None else wsb[wname]
                if mode == "kouter":
                    p = pps.tile([128, Mc * 128], F32, tag="acc_ps", bufs=2)
                    for kc in range(Kc):
                        r = rhs_fn(kc)
                        for mc in range(Mc):
                            nc.tensor.matmul(
                                p[:, mc * 128:(mc + 1) * 128],
                                w[:, kc, mc * 128:(mc + 1) * 128], r,
                                start=(kc == 0), stop=(kc == Kc - 1),
                                skip_group_check=True)
                    consume(p, 0, Mc)
                    return
                for m0 in range(0, Mc, group):
                    g = min(group, Mc - m0)
                    p = pps.tile([128, g * 128], F32, tag="mmps")
                    for sub in range(g):
                        mc = m0 + sub
                        for kc in range(Kc):
                            nc.tensor.matmul(
                                p[:, sub * 128:(sub + 1) * 128],
                                w[:, kc, mc * 128:(mc + 1) * 128],
                                rhs_fn(kc),
                                start=(kc == 0), stop=(kc == Kc - 1))
                    consume(p, m0, g)

            # persistent state
            Hf = cpool.tile([128, KC_D, ROWS], F32, tag="Hf")
            Hs = [cpool.tile([128, KC_D, ROWS], BF16, tag=f"Hs{i}", name=f"Hs{i}")
                  for i in range(2)]

            # ---------- precompute EG = embT @ Wg_top ----------
            EG = cpool.tile([128, KC_D, ROWS], F32, tag="EG")
            TILE_NAMES.clear()
            for _n, _t in (("EG", EG), ("Hf", Hf), ("Hs0", Hs[0]), ("Hs1", Hs[1]),
                           ("embT", embT), ("embTbf", embTbf)):
                TILE_NAMES[_n] = _t.name

            # PE warm-up: ~40 dense matmuls so the HAM un-throttles the PE
            # clock (1.2 -> 2.4 GHz) before the sweeps start. Data irrelevant.
            wu_ps = sps.tile([128, 512], F32, tag="wu_ps", bufs=1)
            for i in range(12):
                nc.tensor.matmul(wu_ps[:], wsb["V0"][:, 0, 0:128],
                                 wsb["V0"][:, 1, 0:512], start=True, stop=True)
            wu_sb = cpool.tile([128, 4], F32, tag="wu_sb")
            nc.vector.tensor_copy(wu_sb[:], wu_ps[:, 0:4])
            nc.sync.dma_start(warm_ext[:], wu_sb[:])

            def eg_consume(p, m0, g):
                nc.vector.tensor_copy(EG[:, m0:m0 + g, :], p[:])
            mm_layer("Wgt", KC_D, KC_D, lambda kc: embTbf[:, kc, :], eg_consume)

            # ---------- Picard sweeps ----------
            for it in range(n_iters):
                first = (it == 0)
                cur = Hs[it % 2]       # shifted H input for this sweep (bf16)
                nxt = Hs[(it + 1) % 2]

                # CTX (bf16): emb + alpha * (Hs @ R)
                if first:
                    CTX = embTbf
                else:
                    CTX = apool.tile([128, KC_D, ROWS], BF16, tag="CTX", bufs=2)

                    def ctx_consume(p, m0, g):
                        nc.vector.scalar_tensor_tensor(
                            CTX[:, m0:m0 + g, :], p[:], ALPHA,
                            embT[:, m0:m0 + g, :], ALU.mult, ALU.add)
                    mm_layer("R", KC_D, KC_D, lambda kc: cur[:, kc, :], ctx_consume)

                # A' = ctx@V0 -> x*(1+erf(x/sqrt2))  (2*gelu; 0.5 folded into V1)
                Abf = apool.tile([128, KC_H, ROWS], BF16, tag="Abf")
                if it == 0:
                    TILE_NAMES["CTX0"] = CTX.name
                    TILE_NAMES["Abf0"] = Abf.name

                def a_consume(p, m0, g):
                    ebuf = apool.tile([128, g * 128], F32, tag=f"erfA{m0 % 8}")
                    nc.scalar.activation(ebuf[:], p[:], AF.Erf, scale=INV_SQRT2)
                    nc.vector.scalar_tensor_tensor(
                        Abf[:, m0:m0 + g, :], ebuf[:], 1.0, p[:],
                        ALU.add, ALU.mult)
                mm_layer("V0", KC_D, KC_H, lambda kc: CTX[:, kc, :], a_consume)

                # TGT' = A'@(V1/2) -> 2*gelu  (scale irrelevant: l2-normalized next)
                TGTbf = apool.tile([128, KC_D, ROWS], BF16, tag="TGTbf", bufs=2)
                if it == 0:
                    TILE_NAMES["TGTbf0"] = TGTbf.name

                def t_consume(p, m0, g):
                    ebuf = apool.tile([128, g * 128], F32, tag=f"erfT{m0 % 8}")
                    nc.scalar.activation(ebuf[:], p[:], AF.Erf, scale=INV_SQRT2)
                    nc.vector.scalar_tensor_tensor(
                        TGTbf[:, m0:m0 + g, :], ebuf[:], 1.0, p[:],
                        ALU.add, ALU.mult)
                mm_layer("V1", KC_H, KC_D, lambda kc: Abf[:, kc, :], t_consume)

                # TF matmuls early: only need `cur`; they fill the PE gap
                # while the l2n chain runs. Consumed later in hp_consume.
                tf_ps = []
                if not first:
                    mm_layer("RWt", KC_D, KC_D, lambda kc: cur[:, kc, :],
                             lambda p, m0, g: tf_ps.append((p, m0, g)))

                # CB = l2n(TGT'): row norms via ones-matmul over partitions+chunks
                sq = npool.tile([128, KC_D, ROWS], BF16, tag="sq")
                nc.vector.tensor_tensor(sq[:], TGTbf[:], TGTbf[:], ALU.mult)
                ssp = sps.tile([1, ROWS], F32, tag="sum_ps")
                for kc in range(KC_D):
                    nc.tensor.matmul(ssp[:], ones_col_bf[:], sq[:, kc, :],
                                     start=(kc == 0), stop=(kc == KC_D - 1))
                ss = npool.tile([1, ROWS], F32, tag="ss")
                nc.vector.tensor_scalar(ss[:], ssp[:], 1e-24, None, ALU.add)
                r_l2 = _nr_rsqrt(nc, npool, ss[:], ROWS, "l2n", iters=2)
                rb_p = pps.tile([128, ROWS], F32, tag="mmps")
                nc.tensor.matmul(rb_p[:], ones_row_f[:], r_l2[:], start=True, stop=True)
                CBbf = apool.tile([128, KC_D, ROWS], BF16, tag="CBbf")
                if it == 0:
                    TILE_NAMES["CBbf0"] = CBbf.name
                for kc in range(KC_D):
                    nc.vector.tensor_tensor(CBbf[:, kc, :], TGTbf[:, kc, :], rb_p[:],
                                            ALU.mult)

                # U' = CB@W1 -> 2*gelu (0.5 folded into W2)
                Ubf = apool.tile([128, KC_D, ROWS], BF16, tag="Ubf", bufs=2)
                if it == 0:
                    TILE_NAMES["Ubf0"] = Ubf.name

                def u_consume(p, m0, g):
                    ebuf = apool.tile([128, g * 128], F32, tag=f"erfU{m0 % 8}")
                    nc.scalar.activation(ebuf[:], p[:], AF.Erf, scale=INV_SQRT2)
                    nc.vector.scalar_tensor_tensor(
                        Ubf[:, m0:m0 + g, :], ebuf[:], 1.0, p[:],
                        ALU.add, ALU.mult)
                mm_layer("W1", KC_D, KC_D, lambda kc: CBbf[:, kc, :], u_consume)

                # CF = U@(W2/2)
                CFbf = apool.tile([128, KC_D, ROWS], BF16, tag="CFbf", bufs=2)
                if it == 0:
                    TILE_NAMES["CFbf0"] = CFbf.name

                def cf_consume(p, m0, g):
                    nc.vector.tensor_copy(CFbf[:, m0:m0 + g, :], p[:])
                mm_layer("W2", KC_D, KC_D, lambda kc: Ubf[:, kc, :], cf_consume)

                # G = sigmoid(EG + CF@Wg_bot); then h_pre and LN, chunk-group-wise
                Gsb = apool.tile([128, KC_D, ROWS], F32, tag="Gsb")
                if it == 0:
                    TILE_NAMES["Gsb0"] = Gsb.name

                def g_consume(p, m0, g):
                    gin = apool.tile([128, g * 128], F32, tag=f"gin{m0 % 8}")
                    nc.vector.tensor_tensor(gin[:], p[:], EG[:, m0:m0 + g, :], ALU.add)
                    nc.scalar.activation(Gsb[:, m0:m0 + g, :], gin[:], AF.Sigmoid)
                mm_layer("W2Wg", KC_D, KC_D, lambda kc: Ubf[:, kc, :], g_consume)

                # h_pre = G*(CF + alpha*TF - EMB) + EMB,  TF = Hs @ RWt
                hpre = apool.tile([128, KC_D, ROWS], F32, tag="hpre")
                if it == 0:
                    TILE_NAMES["hpre0"] = hpre.name

                def hp_consume(p, m0, g):
                    # p: TF psum (zero on first sweep -> p is None)
                    t1 = apool.tile([128, g * 128], F32, tag=f"t1_{m0 % 8}")
                    if first:
                        nc.vector.tensor_tensor(
                            t1[:], CFbf[:, m0:m0 + g, :], embT[:, m0:m0 + g, :],
                            ALU.subtract)
                    else:
                        nc.vector.scalar_tensor_tensor(
                            t1[:], p[:], ALPHA, CFbf[:, m0:m0 + g, :],
                            ALU.mult, ALU.add)
                        nc.vector.tensor_tensor(
                            t1[:], t1[:], embT[:, m0:m0 + g, :], ALU.subtract)
                    nc.vector.tensor_tensor(t1[:], t1[:], Gsb[:, m0:m0 + g, :],
                                            ALU.mult)
                    nc.vector.tensor_tensor(hpre[:, m0:m0 + g, :], t1[:],
                                            embT[:, m0:m0 + g, :], ALU.add)
                if first:
                    hp_consume(None, 0, KC_D)
                else:
                    for (p, m0, g) in tf_ps:
                        hp_consume(p, m0, g)

                # LayerNorm(h_pre) -> Hf (f32) ; gamma=1, beta=0
                s1p = sps.tile([1, ROWS], F32, tag="sum_ps")
                s2p = sps.tile([1, ROWS], F32, tag="sum_ps")
                hpre_bf = npool.tile([128, KC_D, ROWS], BF16, tag="hpre_bf")
                nc.vector.tensor_copy(hpre_bf[:], hpre[:])
                hsq = npool.tile([128, KC_D, ROWS], BF16, tag="hsq")
                nc.vector.tensor_tensor(hsq[:], hpre[:], hpre[:], ALU.mult)
                for kc in range(KC_D):
                    nc.tensor.matmul(s1p[:], ones_col_bf[:], hpre_bf[:, kc, :],
                                     start=(kc == 0), stop=(kc == KC_D - 1))
                for kc in range(KC_D):
                    nc.tensor.matmul(s2p[:], ones_col_bf[:], hsq[:, kc, :],
                                     start=(kc == 0), stop=(kc == KC_D - 1))
                mrow = npool.tile([1, ROWS], F32, tag="mrow")
                nc.vector.tensor_scalar(mrow[:], s1p[:], 1.0 / D, None, ALU.mult)
                var = npool.tile([1, ROWS], F32, tag="var")
                nc.vector.tensor_tensor(var[:], mrow[:], mrow[:], ALU.mult)
                nc.vector.scalar_tensor_tensor(var[:], s2p[:], 1.0 / D, var[:],
                                               ALU.mult, ALU.subtract)
                nc.vector.tensor_scalar(var[:], var[:], 1e-5, None, ALU.add)
                r_ln = _nr_rsqrt(nc, npool, var[:], ROWS, "ln", iters=2)
                mb_p = pps.tile([128, ROWS], F32, tag="mmps")
                nc.tensor.matmul(mb_p[:], ones_row_f[:], mrow[:], start=True, stop=True)
                rb2_p = pps.tile([128, ROWS], F32, tag="mmps")
                nc.tensor.matmul(rb2_p[:], ones_row_f[:], r_ln[:], start=True, stop=True)
                for kc in range(KC_D):
                    d_ = npool.tile([128, ROWS], F32, tag=f"lnd{kc % 3}", name=f"lnd{kc}")
                    nc.vector.tensor_tensor(d_[:], hpre[:, kc, :], mb_p[:],
                                            ALU.subtract)
                    nc.vector.tensor_tensor(Hf[:, kc, :], d_[:], rb2_p[:], ALU.mult)

                # next sweep's shifted input: cols 4.. from my own rows,
                # cols 0..3 from the (lag-2) halo
                if it + 1 < n_iters:
                    for kc in range(KC_D):
                        nc.vector.tensor_copy(nxt[:, kc, 4:ROWS],
                                              Hf[:, kc, 0:ROWS - 4])
                    # launch my halo for sweep it+2
                    nc.sync.dma_start(halo_in[it][:], Hf[:, :, ROWS - 4:ROWS])
                    nc.gpsimd.collective_compute(
                        "AllGather", ALU.bypass, replica_groups=rg,
                        ins=[halo_in[it][:]], outs=[halo_out[it][:]])
                    # consume halo launched at sweep it-1 (contains H^{it-1} edge)
                    if it >= 1:
                        blocks = npool.tile([128, 8, KC_D * 4], F32, tag="blocks")
                        nc.sync.dma_start(
                            blocks[:],
                            halo_out[it - 1].ap().rearrange("(r p) f -> p r f", p=128))
                        hacc = npool.tile([128, KC_D * 4], F32, tag="hacc")
                        nc.vector.tensor_scalar(hacc[:], blocks[:, 0, :],
                                                sel[:, 0:1], None, ALU.mult)
                        for r in range(1, N_CORES):
                            nc.vector.scalar_tensor_tensor(
                                hacc[:], blocks[:, r, :], sel[:, r:r + 1], hacc[:],
                                ALU.mult, ALU.add)
                        nc.vector.tensor_copy(
                            nxt[:, :, 0:4],
                            hacc[:].rearrange("p (k c) -> p k c", k=KC_D))
                    else:
                        nc.vector.memset(nxt[:, :, 0:4], 0.0)

            # ---------- final AllGather of H ----------
            nc.sync.dma_start(hfin_in[:], Hf[:])
            nc.gpsimd.collective_compute(
                "AllGather", ALU.bypass, replica_groups=rg,
                ins=[hfin_in[:]], outs=[hfin_out[:]])

        # ---------- lm_head: logits^T = Wl^T @ H^T, vocab-sharded ----------
        with (
            tc.tile_pool(name="lmpool", bufs=1) as lmpool,
            tc.tile_pool(name="wlpool", bufs=6) as wlpool,
            tc.tile_pool(name="opool", bufs=4) as opool,
            tc.tile_pool(name="lps", bufs=4, space="PSUM") as lps,
        ):
            Hfull = lmpool.tile([128, KC_D, T * B], F32, tag="Hfull")
            nc.sync.dma_start(
                Hfull[:].rearrange("p k (r c) -> p k r c", r=N_CORES),
                hfin_out.ap().rearrange("(r p) (k c) -> p k r c", p=128, k=KC_D))
            TILE_NAMES["Hfull"] = Hfull.name
            Hr = lmpool.tile([128, KC_D, T * B], F32R, tag="Hr")
            TILE_NAMES["Hr"] = Hr.name
            nc.vector.tensor_copy(Hr[:], Hfull[:])

            NV = VPAD // 128
            for vc in range(NV):
                wl_t = wlpool.tile([128, KC_D, 128], F32, tag="wl")
                nc.sync.dma_start(wl_t[:], wl_ext[vc])
                wl_r = wlpool.tile([128, KC_D, 128], F32R, tag="wlr")
                nc.vector.tensor_copy(wl_r[:], wl_t[:])
                for half in range(2):
                    p = lps.tile([128, 512], F32, tag="lmp")
                    for kc in range(KC_D):
                        nc.tensor.matmul(
                            p[:], wl_r[:, kc, :],
                            Hr[:, kc, half * 512:(half + 1) * 512],
                            start=(kc == 0), stop=(kc == KC_D - 1))
                    osb = opool.tile([128, 512], F32, tag="osb")
                    if half == 0:
                        nc.vector.tensor_copy(osb[:], p[:])
                    else:
                        nc.scalar.copy(osb[:], p[:])
                    nc.sync.dma_start(
                        out_ext[vc * 128:(vc + 1) * 128,
                                half * 512:(half + 1) * 512], osb[:])

    nc.compile()
    return nc


def _get_built(n_iters=N_ITERS):
    if n_iters not in _BUILD_CACHE:
        _BUILD_CACHE[n_iters] = build(n_iters)
    return _BUILD_CACHE[n_iters]


def _prep_in_maps(token_ids, embedding, V0, b0, V1, b1, W1, c1, W2, c2, Wg, bg,
                  Wt, gamma, beta, Wl, R_weight):
    f64 = np.float64
    for z in (b0, b1, c1, c2, bg, beta):
        assert np.count_nonzero(np.asarray(z)) == 0, "nonzero bias unsupported"
    assert np.allclose(np.asarray(gamma), 1.0), "gamma != 1 unsupported"

    tok = np.asarray(token_ids).astype(np.int64)           # [B, T]
    emb = np.asarray(embedding, f64)[tok]                  # [B, T, D]
    emb = emb / np.maximum(np.linalg.norm(emb, axis=-1, keepdims=True), 1e-12)
    rows = emb.transpose(1, 0, 2).reshape(T * B, D)        # row = t*4+b

    bf = ml_dtypes.bfloat16
    wt = {
        "R": _t_layout(np.asarray(R_weight, f64)).astype(bf),
        "V0": _t_layout(np.asarray(V0, f64)).astype(bf),
        "V1": _t_layout(np.asarray(V1, f64) * 0.5).astype(bf),
        "W1": _t_layout(np.asarray(W1, f64)).astype(bf),
        "W2": _t_layout(np.asarray(W2, f64) * 0.5).astype(bf),
        "RWt": _t_layout(np.asarray(R_weight, f64) @ np.asarray(Wt, f64)).astype(bf),
        "Wgt": _t_layout(np.asarray(Wg, f64)[:D]).astype(bf),
        "W2Wg": _t_layout(np.asarray(W2, f64) * 0.5 @ np.asarray(Wg, f64)[D:]).astype(bf),
    }
    wl_f32 = np.asarray(Wl, np.float32)

    in_maps = []
    for c in range(N_CORES):
        block = rows[c * ROWS:(c + 1) * ROWS].T            # [D, 128]
        embT = np.ascontiguousarray(
            block.reshape(KC_D, 128, ROWS).transpose(1, 0, 2)).astype(np.float32)
        sel = np.zeros((128, 8), np.float32)
        if c > 0:
            sel[:, c - 1] = 1.0
        wl_shard_cols = np.zeros((D, VPAD), np.float32)
        lo = c * VSHARD
        hi = min(V, lo + VSHARD)
        wl_shard_cols[:, :hi - lo] = wl_f32[:, lo:hi]
        wl_shard = _t_layout(wl_shard_cols)                 # [128, KC_D, VPAD]
        wl_shard = np.ascontiguousarray(
            wl_shard.reshape(128, KC_D, VPAD // 128, 128).transpose(2, 0, 1, 3))
        m = {"embT": embT, "sel": sel, "wl": wl_shard}
        for name, w in wt.items():
            m[f"wb_{name}"] = w
        in_maps.append(m)
    return in_maps


def kernel(**inputs):
    global LAST_RESULT
    in_maps = _prep_in_maps(**{k: np.asarray(v) for k, v in inputs.items()})
    nc = _get_built()
    trace = bool(os.environ.get("KERNEL_TRACE"))
    res = run_bass_kernel_spmd(nc, in_maps, core_ids=list(range(N_CORES)),
                               trace=trace)
    LAST_RESULT = res
    parts = [res.results[c]["out"][:VSHARD] for c in range(N_CORES)]
    L = np.concatenate(parts, axis=0)[:V]                  # [V, T*B]
    out = np.ascontiguousarray(
        L.reshape(V, T, B).transpose(2, 1, 0)).astype(np.float32)
    return out


if __name__ == "__main__":
    pass



# revision 16
# speedup vs baseline: 1.0673x; 1.0673x over previous
"""Trainium2 Bass kernel for nn_AgnisV5 (B=4, T=256, V=50257, D=768, H=3072).

Strategy
--------
The reference is a 256-step sequential recurrence over h (LayerNorm'd each
step) plus a big lm_head projection that does not feed back. The recurrence
map is contractive (Jacobian norm ~0.65), so instead of stepping 256 times
with tiny (M=4) matmuls, we solve the whole sequence by 12 batched Picard
sweeps: H <- StepAll(shift(H)), each sweep a full-width (M=128/core) pass
over all timesteps. bf16 sweeps floor at ~2.8e-3 of logits scale; 12 sweeps
measure ~1.1e-2 of scale (gate 2e-2).

Sharding: time-sharded across 8 cores (128 rows = 32 timesteps x batch 4 per
core), weights replicated in bf16 SBUF-resident form. Cross-core traffic is a
tiny per-sweep boundary halo (lag-2, fully overlapped AllGather) plus a final
2-chunk AllGather of H (bf16) for the vocab-sharded bf16 lm_head, overlapped
with the first lm_head phase.

Keeping the PE dense (v2 restructure):
  - l2n fold: U = gelu2(l2n(TGT)@W1) = gelu2(s*(TGT@W1)) -- W1 runs on raw
    TGT while the norm chain computes s on DVE.
  - LN fold: LN(hpre)@R = s*(hpre@R - m*colsum(R)) -- next sweep's R/RWt
    matmuls run on pre-LN hpre; LN stats apply as a DVE correction. The halo
    ships pre-transformed alpha*s*(P - m*csR) edges.
  - gelu(x) = 0.5*x*(1+erf(x/sqrt(2))): erf on ScalarE, 0.5 folded into the
    next weight matrix on the host; temporal_feat folded as R@Wt on host;
    rsqrt via DVE Newton iterations (no ACT table switch).
"""
import sys, os
sys.path.insert(0, '/opt/trn_rl_repo')
import numpy as np
import ml_dtypes

import concourse.bass as bass
import concourse.bacc as bacc
import concourse.mybir as mybir
import concourse.tile as tile
from concourse.bass_utils import run_bass_kernel_spmd


def _ensure_ntff_hook():
    """The agent image's antenv lacks axon_hooks, which silently disables
    NTFF profiling (exec_time_ns). Shim the module and register the
    ctypes-based hook from trn_agent_boot if available."""
    import types
    if "antenv.axon_hooks" in sys.modules:
        return
    try:
        import antenv
        m = types.ModuleType("antenv.axon_hooks")
        _h = [None]
        m.set_axon_ntff_profile_hook = lambda h: _h.__setitem__(0, h)
        m.get_axon_ntff_profile_hook = lambda: _h[0]
        sys.modules["antenv.axon_hooks"] = m
        antenv.axon_hooks = m
        from trn_agent_boot.trn_boot import _ntff_profile_via_ctypes
        hook = _ntff_profile_via_ctypes("/opt/axon/libaxon_pjrt.so")
        if hook is not None:
            m.set_axon_ntff_profile_hook(hook)
    except Exception:
        pass


_ensure_ntff_hook()

F32 = mybir.dt.float32
BF16 = mybir.dt.bfloat16
AF = mybir.ActivationFunctionType
ALU = mybir.AluOpType

N_CORES = 8
B, T, V, D, H = 4, 256, 50257, 768, 3072
ROWS = 128                 # rows per core = 32 timesteps x 4 batch
KC_D = D // 128            # 6 chunks of the d dimension
KC_H = H // 128            # 24 chunks of the hidden dimension
VPAD = 6400                # per-core vocab shard cols, padded to 50*128
VSHARD = 6283              # ceil(V / 8); host pads vocab to 8*VSHARD = 50264
NV = VPAD // 128           # 50 vocab chunks per core
HALF = ROWS // 2           # final AllGather row-chunk size
N_ITERS = 12
ALPHA = 0.4
INV_SQRT2 = 0.7071067811865476
N_WL_DMA = 10              # wl shard arrives in this many DMAs
WL_PACE = 4                # dummy-MM pace points on the first K wl DMAs

LAST_RESULT = None         # BassKernelResults of the most recent run (for test.py)
TILE_NAMES = {}            # debug: logical name -> sim tensor name

_BUILD_CACHE = {}


def _t_layout(w):
    """[K, M] row-major -> [128, K/128, M] T-layout for stationary lhsT tiles."""
    K, M = w.shape
    assert K % 128 == 0
    return np.ascontiguousarray(w.reshape(K // 128, 128, M).transpose(1, 0, 2))


def _nr_rsqrt(nc, pool, s_ap, n_free, name, iters=2):
    """rsqrt(s) on DVE: bit-trick seed + Newton iterations. s_ap: [1, n] f32."""
    bits = pool.tile([1, n_free], mybir.dt.int32, tag=f"{name}_bits")
    nc.vector.tensor_scalar(bits[:], s_ap.bitcast(mybir.dt.int32), 1, None,
                            ALU.logical_shift_right)
    nc.vector.tensor_scalar(bits[:], bits[:], -1, 0x5f3759df, ALU.mult, ALU.add)
    y = pool.tile([1, n_free], F32, tag=f"{name}_y")
    nc.vector.tensor_copy(y[:], bits[:].bitcast(F32))
    half = pool.tile([1, n_free], F32, tag=f"{name}_half")
    nc.vector.tensor_scalar(half[:], s_ap, 0.5, None, ALU.mult)
    yy = pool.tile([1, n_free], F32, tag=f"{name}_yy")
    e = pool.tile([1, n_free], F32, tag=f"{name}_e")
    for _ in range(iters):
        nc.vector.tensor_tensor(yy[:], y[:], y[:], ALU.mult)
        nc.vector.tensor_tensor(e[:], yy[:], half[:], ALU.mult)
        nc.vector.tensor_scalar(e[:], e[:], -1.0, 1.5, ALU.mult, ALU.add)
        nc.vector.tensor_tensor(y[:], y[:], e[:], ALU.mult)
    return y


def build(n_iters=N_ITERS):
    nc = bacc.Bacc("TRN2", target_bir_lowering=False, debug=False,
                   num_devices=N_CORES)

    # ---- DRAM parameters (per-core data via in_maps) ----
    # DMA issue order matters: embT + Wgt + V0 first so compute starts early.
    embT_ext = nc.declare_dram_parameter("embT", [128, KC_D, ROWS], F32, isOutput=False)
    wb_ext = {}
    for name, (wk, wm) in dict(Wgt=(D, D), V0=(D, H), V1=(H, D), W1=(D, D),
                               W2Wg=(D, D), W2=(D, D), R=(D, D), RWt=(D, D)).items():
        wb_ext[name] = nc.declare_dram_parameter(f"wb_{name}", [128, wk // 128, wm],
                                                 BF16, isOutput=False)
    sel_ext = nc.declare_dram_parameter("sel", [128, 8], F32, isOutput=False)
    csn_ext = nc.declare_dram_parameter("csn", [128, KC_D, 2], F32, isOutput=False)
    wl_ext = nc.declare_dram_parameter("wl", [NV, 128, KC_D, 128], BF16, isOutput=False)
    out_ext = nc.declare_dram_parameter("out", [2, VPAD, N_CORES * HALF], F32,
                                        isOutput=True)
    warm_ext = nc.declare_dram_parameter("warm", [128, 8], F32, isOutput=True)

    # ---- internal DRAM for collectives ----
    halo_in = [nc.dram_tensor(f"halo_in_{k}", [128, KC_D * 8], F32)
               for k in range(n_iters)]
    halo_out = [nc.dram_tensor(f"halo_out_{k}", [N_CORES * 128, KC_D * 8], F32,
                               addr_space="Shared") for k in range(n_iters)]
    ccw_in = nc.dram_tensor("ccw_in", [1, 32], F32)
    ccw_out = nc.dram_tensor("ccw_out", [N_CORES, 32], F32, addr_space="Shared")
    hfa_in = nc.dram_tensor("hfa_in", [128, KC_D * HALF], BF16)
    hfa_out = nc.dram_tensor("hfa_out", [N_CORES * 128, KC_D * HALF], BF16,
                             addr_space="Shared")
    hfb_in = nc.dram_tensor("hfb_in", [128, KC_D * HALF], BF16)
    hfb_out = nc.dram_tensor("hfb_out", [N_CORES * 128, KC_D * HALF], BF16,
                             addr_space="Shared")

    rg = [list(range(N_CORES))]

    with tile.TileContext(nc) as tc:
        with (
            tc.tile_pool(name="wpool", bufs=1) as wpool,
            tc.tile_pool(name="cpool", bufs=1) as cpool,      # constants / persistents
            tc.tile_pool(name="apool", bufs=1) as apool,      # per-iteration activations
            tc.tile_pool(name="npool", bufs=1) as npool,      # norm scratch
            tc.tile_pool(name="pps", bufs=4, space="PSUM") as pps,
            tc.tile_pool(name="sps", bufs=2, space="PSUM") as sps,
        ):
            # ---------- load persistent data (order = DMA priority) ----------
            embT = cpool.tile([128, KC_D, ROWS], F32, tag="embT")
            nc.sync.dma_start(embT[:], embT_ext[:])
            wsb = {}
            for name, ext in wb_ext.items():
                t_ = wpool.tile(list(ext.shape), BF16, tag=f"w_{name}")
                nc.sync.dma_start(t_[:], ext[:])
                wsb[name] = t_
            sel = cpool.tile([128, 8], F32, tag="sel")
            nc.sync.dma_start(sel[:], sel_ext[:])
            csn = cpool.tile([128, KC_D, 2], F32, tag="csn")
            nc.sync.dma_start(csn[:], csn_ext[:])
            embTbf = cpool.tile([128, KC_D, ROWS], BF16, tag="embTbf")
            nc.vector.tensor_copy(embTbf[:], embT[:])
            # warm up the collective path early (first call pays ENCD init)
            nc.sync.dma_start(ccw_in[:], embT[0:1, 0, 0:32])
            nc.gpsimd.collective_compute(
                "AllGather", ALU.bypass, replica_groups=rg,
                ins=[ccw_in[:]], outs=[ccw_out[:]])

            ones_col_bf = cpool.tile([128, 1], BF16, tag="ones_col_bf")
            nc.vector.memset(ones_col_bf[:], 1.0)
            ones_row_f = cpool.tile([1, 128], F32, tag="ones_row_f")
            nc.vector.memset(ones_row_f[:], 1.0)

            # PE warm-up on the first-landing weight (HAM un-throttle).
            wu_ps = sps.tile([128, 512], F32, tag="wu_ps", bufs=1)
            for i in range(12):
                nc.tensor.matmul(wu_ps[:], wsb["Wgt"][:, 0, 0:128],
                                 wsb["Wgt"][:, 1, 0:512], start=True, stop=True)
            wu_sb = cpool.tile([128, 4], F32, tag="wu_sb")
            nc.vector.tensor_copy(wu_sb[:], wu_ps[:, 0:4])
            nc.sync.dma_start(warm_ext[:, 0:4], wu_sb[:])

            # helper: one weight "layer": out chunks [mc] = sum_kc lhsT @ rhs
            def mm_layer(wname, Kc, Mc, rhs_fn, consume, group=4):
                """consume(psum_ap, m0, g) handles [128, g*128] out."""
                w = wsb[wname]
                for m0 in range(0, Mc, group):
                    g = min(group, Mc - m0)
                    p = pps.tile([128, g * 128], F32, tag="mmps")
                    for sub in range(g):
                        mc = m0 + sub
                        for kc in range(Kc):
                            nc.tensor.matmul(
                                p[:, sub * 128:(sub + 1) * 128],
                                w[:, kc, mc * 128:(mc + 1) * 128],
                                rhs_fn(kc),
                                start=(kc == 0), stop=(kc == Kc - 1))
                    consume(p, m0, g)

            # persistent state across sweeps
            embT_ = embT  # alias for closures
            TILE_NAMES.clear()

            # ---------- precompute EG = embT @ Wg_top ----------
            EG = cpool.tile([128, KC_D, ROWS], F32, tag="EG")

            def eg_consume(p, m0, g):
                nc.vector.tensor_copy(EG[:, m0:m0 + g, :], p[:])
            mm_layer("Wgt", KC_D, KC_D, lambda kc: embTbf[:, kc, :], eg_consume)

            # ---------- Picard sweeps ----------
            CTXs = [cpool.tile([128, KC_D, ROWS], BF16, tag=f"CTX{i}",
                               name=f"CTX{i}") for i in range(2)]
            TFs = [cpool.tile([128, KC_D, ROWS], F32, tag=f"TF{i}",
                              name=f"TF{i}") for i in range(2)]
            Hbf = cpool.tile([128, KC_D, ROWS], BF16, tag="Hbf")

            for it in range(n_iters):
                first = (it == 0)
                last = (it == n_iters - 1)
                CTX = embTbf if first else CTXs[it % 2]
                TFc = None if first else TFs[it % 2]
                CTXn = CTXs[(it + 1) % 2]
                TFn = TFs[(it + 1) % 2]

                # A' = ctx@V0 -> x*(1+erf(x/sqrt2))  (2*gelu; 0.5 folded into V1)
                Abf = apool.tile([128, KC_H, ROWS], BF16, tag="Abf")

                def a_consume(p, m0, g):
                    ebuf = apool.tile([128, g * 128], F32, tag=f"erf{m0 % 8}")
                    nc.scalar.activation(ebuf[:], p[:], AF.Erf, scale=INV_SQRT2)
                    nc.vector.scalar_tensor_tensor(
                        Abf[:, m0:m0 + g, :], ebuf[:], 1.0, p[:],
                        ALU.add, ALU.mult)
                mm_layer("V0", KC_D, KC_H, lambda kc: CTX[:, kc, :], a_consume)

                # TGT' = A'@(V1/2) -> 2*gelu; also accumulate sum(TGT^2) rows
                TGTbf = apool.tile([128, KC_D, ROWS], BF16, tag="TGTbf", bufs=2)
                ssp = sps.tile([1, ROWS], F32, tag="sum_ps")
                sq = npool.tile([128, KC_D, ROWS], BF16, tag="sqh", bufs=2)

                def t_consume(p, m0, g):
                    ebuf = apool.tile([128, g * 128], F32, tag=f"erf{m0 % 8}")
                    nc.scalar.activation(ebuf[:], p[:], AF.Erf, scale=INV_SQRT2)
                    nc.vector.scalar_tensor_tensor(
                        TGTbf[:, m0:m0 + g, :], ebuf[:], 1.0, p[:],
                        ALU.add, ALU.mult)
                    nc.vector.tensor_tensor(
                        sq[:, m0:m0 + g, :], TGTbf[:, m0:m0 + g, :],
                        TGTbf[:, m0:m0 + g, :], ALU.mult)
                    for sub in range(g):
                        mc = m0 + sub
                        nc.tensor.matmul(ssp[:], ones_col_bf[:], sq[:, mc, :],
                                         start=(mc == 0), stop=(mc == KC_D - 1))
                mm_layer("V1", KC_H, KC_D, lambda kc: Abf[:, kc, :], t_consume)

                # l2n scale s on DVE (runs under W1's matmuls)
                ss = npool.tile([1, ROWS], F32, tag="ss")
                nc.vector.tensor_scalar(ss[:], ssp[:], 1e-24, None, ALU.add)
                r_l2 = _nr_rsqrt(nc, npool, ss[:], ROWS, "l2n", iters=2)
                rb_p = pps.tile([128, ROWS], F32, tag="mmps")
                nc.tensor.matmul(rb_p[:], ones_row_f[:], r_l2[:], start=True, stop=True)
                rb = npool.tile([128, ROWS], F32, tag="rb")
                nc.vector.tensor_copy(rb[:], rb_p[:])

                # U' = (s*(TGT@W1)) -> 2*gelu (0.5 folded into W2)
                Ubf = apool.tile([128, KC_D, ROWS], BF16, tag="Ubf", bufs=2)

                def u_consume(p, m0, g):
                    ysc = apool.tile([128, g * 128], F32, tag=f"ysc{m0 % 8}")
                    for sub in range(g):
                        nc.vector.tensor_tensor(
                            ysc[:, sub * 128:(sub + 1) * 128],
                            p[:, sub * 128:(sub + 1) * 128], rb[:], ALU.mult)
                    ebuf = apool.tile([128, g * 128], F32, tag=f"erf{m0 % 8}")
                    nc.scalar.activation(ebuf[:], ysc[:], AF.Erf, scale=INV_SQRT2)
                    nc.vector.scalar_tensor_tensor(
                        Ubf[:, m0:m0 + g, :], ebuf[:], 1.0, ysc[:],
                        ALU.add, ALU.mult)
                mm_layer("W1", KC_D, KC_D, lambda kc: TGTbf[:, kc, :], u_consume)

                # G = sigmoid(EG + U@(W2/2@Wg_bot))
                Gsb = apool.tile([128, KC_D, ROWS], F32, tag="Gsb")

                def g_consume(p, m0, g):
                    gin = apool.tile([128, g * 128], F32, tag=f"erf{m0 % 8}")
                    nc.vector.tensor_tensor(gin[:], p[:], EG[:, m0:m0 + g, :], ALU.add)
                    nc.scalar.activation(Gsb[:, m0:m0 + g, :], gin[:], AF.Sigmoid)
                mm_layer("W2Wg", KC_D, KC_D, lambda kc: Ubf[:, kc, :], g_consume)

                # CF = U@(W2/2); hpre = G*(CF + TFc - EMB) + EMB (TFc pre-alpha'd)
                # also LN stat accumulation per chunk
                hpre = apool.tile([128, KC_D, ROWS], F32, tag="hpre")
                hpre_bf = npool.tile([128, KC_D, ROWS], BF16, tag="hpre_bf", bufs=2)
                hsq = npool.tile([128, KC_D, ROWS], BF16, tag="sqh", bufs=2)
                s1p = sps.tile([1, ROWS], F32, tag="sum_ps")
                s2p = sps.tile([1, ROWS], F32, tag="sum_ps")

                def hp_consume(p, m0, g):
                    t1 = apool.tile([128, g * 128], F32, tag=f"t1_{m0 % 8}")
                    if first:
                        nc.vector.tensor_tensor(
                            t1[:], p[:], embT_[:, m0:m0 + g, :], ALU.subtract)
                    else:
                        nc.vector.tensor_tensor(
                            t1[:], p[:], TFc[:, m0:m0 + g, :], ALU.add)
                        nc.vector.tensor_tensor(
                            t1[:], t1[:], embT_[:, m0:m0 + g, :], ALU.subtract)
                    nc.vector.tensor_tensor(t1[:], t1[:], Gsb[:, m0:m0 + g, :],
                                            ALU.mult)
                    nc.vector.tensor_tensor(hpre[:, m0:m0 + g, :], t1[:],
                                            embT_[:, m0:m0 + g, :], ALU.add)
                    nc.vector.tensor_copy(hpre_bf[:, m0:m0 + g, :],
                                          hpre[:, m0:m0 + g, :])
                    nc.vector.tensor_tensor(hsq[:, m0:m0 + g, :],
                                            hpre[:, m0:m0 + g, :],
                                            hpre[:, m0:m0 + g, :], ALU.mult)
                    for sub in range(g):
                        mc = m0 + sub
                        nc.tensor.matmul(s1p[:], ones_col_bf[:], hpre_bf[:, mc, :],
                                         start=(mc == 0), stop=(mc == KC_D - 1))
                        nc.tensor.matmul(s2p[:], ones_col_bf[:], hsq[:, mc, :],
                                         start=(mc == 0), stop=(mc == KC_D - 1))
                mm_layer("W2", KC_D, KC_D, lambda kc: Ubf[:, kc, :], hp_consume)

                # LN stats on DVE (overlap with R/RWt matmuls below)
                mrow = npool.tile([1, ROWS], F32, tag="mrow")
                nc.vector.tensor_scalar(mrow[:], s1p[:], 1.0 / D, None, ALU.mult)
                var = npool.tile([1, ROWS], F32, tag="var")
                nc.vector.tensor_tensor(var[:], mrow[:], mrow[:], ALU.mult)
                nc.vector.scalar_tensor_tensor(var[:], s2p[:], 1.0 / D, var[:],
                                               ALU.mult, ALU.subtract)
                nc.vector.tensor_scalar(var[:], var[:], 1e-5, None, ALU.add)
                r_ln = _nr_rsqrt(nc, npool, var[:], ROWS, "ln", iters=2)
                mb_p = pps.tile([128, ROWS], F32, tag="mmps")
                nc.tensor.matmul(mb_p[:], ones_row_f[:], mrow[:], start=True,
                                 stop=True)

                if last:
                    # apply LN -> Hbf (bf16) for the final AllGather
                    rb2_p = pps.tile([128, ROWS], F32, tag="mmps")
                    nc.tensor.matmul(rb2_p[:], ones_row_f[:], r_ln[:], start=True,
                                     stop=True)
                    for kc in range(KC_D):
                        d_ = npool.tile([128, ROWS], F32, tag=f"lnd{kc % 3}",
                                        name=f"lnd{kc}")
                        nc.vector.tensor_tensor(d_[:], hpre[:, kc, :], mb_p[:],
                                                ALU.subtract)
                        nc.vector.tensor_tensor(Hbf[:, kc, :], d_[:], rb2_p[:],
                                                ALU.mult)
                    continue

                # alpha*s broadcast
                as_row = npool.tile([1, ROWS], F32, tag="as_row")
                nc.vector.tensor_scalar(as_row[:], r_ln[:], ALPHA, None, ALU.mult)
                asb_p = pps.tile([128, ROWS], F32, tag="mmps")
                nc.tensor.matmul(asb_p[:], ones_row_f[:], as_row[:], start=True,
                                 stop=True)
                asb = npool.tile([128, ROWS], F32, tag="asb")
                nc.vector.tensor_copy(asb[:], asb_p[:])
                mb = npool.tile([128, ROWS], F32, tag="mb")
                nc.vector.tensor_copy(mb[:], mb_p[:])

                # P = hpre@R: CTXn interior = alpha*s*(P - m*csR) + emb
                Ppre = apool.tile([128, KC_D, ROWS], F32, tag="Ppre")

                def p_consume(p, m0, g):
                    for sub in range(g):
                        mc = m0 + sub
                        sl = slice(sub * 128, (sub + 1) * 128)
                        tmp = apool.tile([128, ROWS], F32, tag=f"ptmp{mc % 4}")
                        # tmp = P - m*csR   (csn[:, :, 0] holds -colsum(R))
                        nc.vector.scalar_tensor_tensor(
                            tmp[:], mb[:], csn[:, mc, 0:1], p[:, sl],
                            ALU.mult, ALU.add)
                        nc.vector.tensor_tensor(Ppre[:, mc, :], tmp[:], asb[:],
                                                ALU.mult)
                        nc.vector.tensor_tensor(
                            CTXn[:, mc, 4:ROWS], Ppre[:, mc, 0:ROWS - 4],
                            embT_[:, mc, 4:ROWS], ALU.add)
                mm_layer("R", KC_D, KC_D, lambda kc: hpre_bf[:, kc, :], p_consume)

                # Q = hpre@RWt: TFn interior (shifted) = alpha*s*(Q - m*csW)
                hQe = apool.tile([128, KC_D, 4], F32, tag="hQe", bufs=2)

                def q_consume(p, m0, g):
                    for sub in range(g):
                        mc = m0 + sub
                        sl = slice(sub * 128, (sub + 1) * 128)
                        tmp = apool.tile([128, ROWS], F32, tag=f"qtmp{mc % 4}")
                        nc.vector.scalar_tensor_tensor(
                            tmp[:], mb[:], csn[:, mc, 1:2], p[:, sl],
                            ALU.mult, ALU.add)
                        nc.vector.tensor_tensor(
                            TFn[:, mc, 4:ROWS], tmp[:, 0:ROWS - 4],
                            asb[:, 0:ROWS - 4], ALU.mult)
                        nc.vector.tensor_tensor(
                            hQe[:, mc, :], tmp[:, ROWS - 4:ROWS],
                            asb[:, ROWS - 4:ROWS], ALU.mult)
                mm_layer("RWt", KC_D, KC_D, lambda kc: hpre_bf[:, kc, :], q_consume)

                # launch my halo (P edge + Q edge) for sweep it+2
                if it + 2 < n_iters:
                    hio = halo_in[it].ap().rearrange("p (k c) -> p k c", k=KC_D)
                    nc.sync.dma_start(hio[:, :, 0:4], Ppre[:, :, ROWS - 4:ROWS])
                    nc.sync.dma_start(hio[:, :, 4:8], hQe[:])
                    nc.gpsimd.collective_compute(
                        "AllGather", ALU.bypass, replica_groups=rg,
                        ins=[halo_in[it][:]], outs=[halo_out[it][:]])

                # boundary rows 0:4 of CTXn/TFn for sweep it+1:
                if it >= 1:
                    # consume halo launched at sweep it-1 (lag-2)
                    blocks = npool.tile([128, 8, KC_D * 8], F32, tag="blocks")
                    nc.sync.dma_start(
                        blocks[:],
                        halo_out[it - 1].ap().rearrange("(r p) f -> p r f", p=128))
                    hacc = npool.tile([128, KC_D * 8], F32, tag="hacc")
                    nc.vector.tensor_scalar(hacc[:], blocks[:, 0, :],
                                            sel[:, 0:1], None, ALU.mult)
                    for r in range(1, N_CORES):
                        nc.vector.scalar_tensor_tensor(
                            hacc[:], blocks[:, r, :], sel[:, r:r + 1], hacc[:],
                            ALU.mult, ALU.add)
                    ha = hacc[:].rearrange("p (k c) -> p k c", k=KC_D)
                    for kc in range(KC_D):
                        nc.vector.tensor_tensor(
                            CTXn[:, kc, 0:4], ha[:, kc, 0:4],
                            embT_[:, kc, 0:4], ALU.add)
                    nc.vector.tensor_copy(TFn[:, :, 0:4], ha[:, :, 4:8])
                else:
                    # sweep 1 boundary: h_prev = 0
                    for kc in range(KC_D):
                        nc.vector.tensor_copy(CTXn[:, kc, 0:4], embT_[:, kc, 0:4])
                    nc.vector.memset(TFn[:, :, 0:4], 0.0)

            # ---------- final 2-chunk AllGather of Hbf ----------
            nc.sync.dma_start(hfa_in.ap().rearrange("p (k c) -> p k c", k=KC_D),
                              Hbf[:, :, 0:HALF])
            nc.gpsimd.collective_compute(
                "AllGather", ALU.bypass, replica_groups=rg,
                ins=[hfa_in[:]], outs=[hfa_out[:]])
            nc.sync.dma_start(hfb_in.ap().rearrange("p (k c) -> p k c", k=KC_D),
                              Hbf[:, :, HALF:ROWS])
            nc.gpsimd.collective_compute(
                "AllGather", ALU.bypass, replica_groups=rg,
                ins=[hfb_in[:]], outs=[hfb_out[:]])

        # ---------- lm_head: logits^T = Wl^T @ H^T, vocab-sharded, bf16 ----------
        with (
            tc.tile_pool(name="lmpool", bufs=1) as lmpool,
            tc.tile_pool(name="opool", bufs=4) as opool,
            tc.tile_pool(name="lps", bufs=4, space="PSUM") as lps,
            tc.tile_pool(name="dps", bufs=1, space="PSUM") as dps,
        ):
            # whole Wl shard SBUF-resident (bf16, 9.8MB), in N_WL_DMA chunk tiles
            per = NV // N_WL_DMA
            wlt = []
            du = dps.tile([128, 128], F32, tag="du_ps")
            for i in range(N_WL_DMA):
                t_ = lmpool.tile([128, per * KC_D, 128], BF16, tag=f"wl{i}",
                                 name=f"wl{i}")
                nc.sync.dma_start(
                    t_[:].rearrange("p (v k) c -> p v k c", v=per),
                    wl_ext[i * per:(i + 1) * per].rearrange("v p k c -> p v k c"))
                wlt.append(t_)
                if i < WL_PACE:
                    # paced dummy matmuls: keep the PE warm through the AG gap
                    for j in range(4):
                        nc.tensor.matmul(du[:], t_[:, j, :], t_[:, j + 1, :],
                                         start=True, stop=True)
            wu_sb2 = opool.tile([128, 4], F32, tag="wu_sb2")
            nc.vector.tensor_copy(wu_sb2[:], du[:, 0:4])
            nc.sync.dma_start(warm_ext[:, 4:8], wu_sb2[:])

            Hra = lmpool.tile([128, KC_D, N_CORES * HALF], BF16, tag="Hra")
            nc.sync.dma_start(
                Hra[:].rearrange("p k (r c) -> p k r c", r=N_CORES),
                hfa_out.ap().rearrange("(r p) (k c) -> p k r c", p=128, k=KC_D))
            Hrb = lmpool.tile([128, KC_D, N_CORES * HALF], BF16, tag="Hrb")
            nc.sync.dma_start(
                Hrb[:].rearrange("p k (r c) -> p k r c", r=N_CORES),
                hfb_out.ap().rearrange("(r p) (k c) -> p k r c", p=128, k=KC_D))

            for half, Hr in ((0, Hra), (1, Hrb)):
                for vc in range(NV):
                    p = lps.tile([128, N_CORES * HALF], F32, tag="lmp")
                    wt_ = wlt[vc // per]
                    for kc in range(KC_D):
                        nc.tensor.matmul(
                            p[:], wt_[:, (vc % per) * KC_D + kc, :], Hr[:, kc, :],
                            start=(kc == 0), stop=(kc == KC_D - 1))
                    osb = opool.tile([128, N_CORES * HALF], F32, tag="osb")
                    if vc % 2 == 0:
                        nc.vector.tensor_copy(osb[:], p[:])
                    else:
                        nc.scalar.copy(osb[:], p[:])
                    nc.sync.dma_start(
                        out_ext[half, vc * 128:(vc + 1) * 128, :], osb[:])

    nc.compile()
    return nc


def _get_built(n_iters=N_ITERS):
    if n_iters not in _BUILD_CACHE:
        _BUILD_CACHE[n_iters] = build(n_iters)
    return _BUILD_CACHE[n_iters]


def _prep_in_maps(token_ids, embedding, V0, b0, V1, b1, W1, c1, W2, c2, Wg, bg,
                  Wt, gamma, beta, Wl, R_weight):
    f64 = np.float64
    for z in (b0, b1, c1, c2, bg, beta):
        assert np.count_nonzero(np.asarray(z)) == 0, "nonzero bias unsupported"
    assert np.allclose(np.asarray(gamma), 1.0), "gamma != 1 unsupported"

    tok = np.asarray(token_ids).astype(np.int64)           # [B, T]
    emb = np.asarray(embedding, f64)[tok]                  # [B, T, D]
    emb = emb / np.maximum(np.linalg.norm(emb, axis=-1, keepdims=True), 1e-12)
    rows = emb.transpose(1, 0, 2).reshape(T * B, D)        # row = t*4+b

    bf = ml_dtypes.bfloat16
    R_bf = _t_layout(np.asarray(R_weight, f64)).astype(bf)
    RWt_bf = _t_layout(np.asarray(R_weight, f64) @ np.asarray(Wt, f64)).astype(bf)
    wt = {
        "R": R_bf,
        "V0": _t_layout(np.asarray(V0, f64)).astype(bf),
        "V1": _t_layout(np.asarray(V1, f64) * 0.5).astype(bf),
        "W1": _t_layout(np.asarray(W1, f64)).astype(bf),
        "W2": _t_layout(np.asarray(W2, f64) * 0.5).astype(bf),
        "RWt": RWt_bf,
        "Wgt": _t_layout(np.asarray(Wg, f64)[:D]).astype(bf),
        "W2Wg": _t_layout(np.asarray(W2, f64) * 0.5 @ np.asarray(Wg, f64)[D:]).astype(bf),
    }
    # negated col-sums of the bf16-rounded R / RWt, in T-layout [128, KC_D]
    csn = np.zeros((128, KC_D, 2), np.float32)
    for i, w_bf in enumerate((R_bf, RWt_bf)):
        cs = -w_bf.astype(f64).sum(axis=(0, 1))            # [-colsum] over K
        csn[:, :, i] = cs.reshape(KC_D, 128).T
    wl_bf = np.asarray(Wl, f64).astype(bf)

    in_maps = []
    for c in range(N_CORES):
        block = rows[c * ROWS:(c + 1) * ROWS].T            # [D, 128]
        embT = np.ascontiguousarray(
            block.reshape(KC_D, 128, ROWS).transpose(1, 0, 2)).astype(np.float32)
        sel = np.zeros((128, 8), np.float32)
        if c > 0:
            sel[:, c - 1] = 1.0
        wl_shard_cols = np.zeros((D, VPAD), bf)
        lo = c * VSHARD
        hi = min(V, lo + VSHARD)
        wl_shard_cols[:, :hi - lo] = wl_bf[:, lo:hi]
        wl_shard = _t_layout(wl_shard_cols)                 # [128, KC_D, VPAD]
        wl_shard = np.ascontiguousarray(
            wl_shard.reshape(128, KC_D, NV, 128).transpose(2, 0, 1, 3))
        m = {"embT": embT, "sel": sel, "csn": csn, "wl": wl_shard}
        for name, w in wt.items():
            m[f"wb_{name}"] = w
        in_maps.append(m)
    return in_maps


def kernel(**inputs):
    global LAST_RESULT
    in_maps = _prep_in_maps(**{k: np.asarray(v) for k, v in inputs.items()})
    nc = _get_built()
    trace = bool(os.environ.get("KERNEL_TRACE"))
    res = run_bass_kernel_spmd(nc, in_maps, core_ids=list(range(N_CORES)),
                               trace=trace)
    LAST_RESULT = res
    parts = []
    for c in range(N_CORES):
        o = res.results[c]["out"]                          # [2, VPAD, 8*HALF]
        o = o.reshape(2, VPAD, N_CORES, HALF)
        o = o.transpose(1, 2, 0, 3).reshape(VPAD, T * B)   # row = core*128+half*64+i
        parts.append(o[:VSHARD])
    L = np.concatenate(parts, axis=0)[:V]                  # [V, T*B]
    out = np.ascontiguousarray(
        L.reshape(V, T, B).transpose(2, 1, 0)).astype(np.float32)
    return out


if __name__ == "__main__":
    pass


# revision 17
# speedup vs baseline: 1.0679x; 1.0006x over previous
"""Trainium2 Bass kernel for nn_AgnisV5 (B=4, T=256, V=50257, D=768, H=3072).

Strategy
--------
The reference is a 256-step sequential recurrence over h (LayerNorm'd each
step) plus a big lm_head projection that does not feed back. The recurrence
map is contractive (Jacobian norm ~0.65), so instead of stepping 256 times
with tiny (M=4) matmuls, we solve the whole sequence by 12 batched Picard
sweeps: H <- StepAll(shift(H)), each sweep a full-width (M=128/core) pass
over all timesteps. bf16 sweeps floor at ~2.8e-3 of logits scale; 12 sweeps
measure ~1.2e-2 of scale (gate 2e-2).

Sharding: time-sharded across 8 cores (128 rows = 32 timesteps x batch 4 per
core), weights replicated in bf16 SBUF-resident form. Cross-core traffic is a
tiny per-sweep boundary halo (lag-2, fully overlapped AllGather) plus a final
2-chunk AllGather of H (bf16) for the vocab-sharded bf16 lm_head, overlapped
with the first lm_head phase.

Keeping the PE dense (v3):
  - All gelus via the exact-gelu ACT table; sigmoid(x) = 0.5+0.5*tanh(x/2)
    keeps the gate in the same table set (no ACT table switches). Consumes
    become a single ScalarE op reading PSUM directly.
  - l2n fold: U = gelu(l2n(TGT)@W1) = gelu(s*(TGT@W1)) -- W1 runs on raw
    TGT while the norm chain computes s on DVE; the s-broadcast matmul is
    placed mid-W1 so it never stalls the in-order PE queue.
  - LN fold: LN(hpre)@R = s*(hpre@R - m*colsum(R)) -- next sweep's R/RWt
    matmuls run on pre-LN hpre; LN stats apply as a DVE correction with
    deferred psum consumes, stat/broadcast matmuls slotted between R/RWt
    groups. The halo ships pre-transformed alpha*s*(P - m*csR) edges.
  - V0/V1 live in their own tile pool so their SBUF frees at last use,
    letting the lm_head's Wl stream start during the final sweep.
  - rsqrt via DVE Newton iterations (no ACT table switch).
"""
import sys, os
sys.path.insert(0, '/opt/trn_rl_repo')
import numpy as np
import ml_dtypes

import concourse.bass as bass
import concourse.bacc as bacc
import concourse.mybir as mybir
import concourse.tile as tile
from concourse.bass_utils import run_bass_kernel_spmd


def _ensure_ntff_hook():
    """The agent image's antenv lacks axon_hooks, which silently disables
    NTFF profiling (exec_time_ns). Shim the module and register the
    ctypes-based hook from trn_agent_boot if available."""
    import types
    if "antenv.axon_hooks" in sys.modules:
        return
    try:
        import antenv
        m = types.ModuleType("antenv.axon_hooks")
        _h = [None]
        m.set_axon_ntff_profile_hook = lambda h: _h.__setitem__(0, h)
        m.get_axon_ntff_profile_hook = lambda: _h[0]
        sys.modules["antenv.axon_hooks"] = m
        antenv.axon_hooks = m
        from trn_agent_boot.trn_boot import _ntff_profile_via_ctypes
        hook = _ntff_profile_via_ctypes("/opt/axon/libaxon_pjrt.so")
        if hook is not None:
            m.set_axon_ntff_profile_hook(hook)
    except Exception:
        pass


_ensure_ntff_hook()

F32 = mybir.dt.float32
BF16 = mybir.dt.bfloat16
AF = mybir.ActivationFunctionType
ALU = mybir.AluOpType

N_CORES = 8
B, T, V, D, H = 4, 256, 50257, 768, 3072
ROWS = 128                 # rows per core = 32 timesteps x 4 batch
KC_D = D // 128            # 6 chunks of the d dimension
KC_H = H // 128            # 24 chunks of the hidden dimension
VPAD = 6400                # per-core vocab shard cols, padded to 50*128
VSHARD = 6283              # ceil(V / 8); host pads vocab to 8*VSHARD = 50264
NV = VPAD // 128           # 50 vocab chunks per core
HALF = ROWS // 2           # final AllGather row-chunk size
N_ITERS = 12
ALPHA = 0.4

LAST_RESULT = None         # BassKernelResults of the most recent run (for test.py)
TILE_NAMES = {}

_BUILD_CACHE = {}


def _t_layout(w):
    """[K, M] row-major -> [128, K/128, M] T-layout for stationary lhsT tiles."""
    K, M = w.shape
    assert K % 128 == 0
    return np.ascontiguousarray(w.reshape(K // 128, 128, M).transpose(1, 0, 2))


def _nr_rsqrt(nc, pool, s_ap, n_free, name, iters=2):
    """rsqrt(s) on DVE: bit-trick seed + Newton iterations. s_ap: [1, n] f32."""
    bits = pool.tile([1, n_free], mybir.dt.int32, tag=f"{name}_bits")
    nc.vector.tensor_scalar(bits[:], s_ap.bitcast(mybir.dt.int32), 1, None,
                            ALU.logical_shift_right)
    nc.vector.tensor_scalar(bits[:], bits[:], -1, 0x5f3759df, ALU.mult, ALU.add)
    y = pool.tile([1, n_free], F32, tag=f"{name}_y")
    nc.vector.tensor_copy(y[:], bits[:].bitcast(F32))
    half = pool.tile([1, n_free], F32, tag=f"{name}_half")
    nc.vector.tensor_scalar(half[:], s_ap, 0.5, None, ALU.mult)
    yy = pool.tile([1, n_free], F32, tag=f"{name}_yy")
    e = pool.tile([1, n_free], F32, tag=f"{name}_e")
    for _ in range(iters):
        nc.vector.tensor_tensor(yy[:], y[:], y[:], ALU.mult)
        nc.vector.tensor_tensor(e[:], yy[:], half[:], ALU.mult)
        nc.vector.tensor_scalar(e[:], e[:], -1.0, 1.5, ALU.mult, ALU.add)
        nc.vector.tensor_tensor(y[:], y[:], e[:], ALU.mult)
    return y


def build(n_iters=N_ITERS):
    nc = bacc.Bacc("TRN2", target_bir_lowering=False, debug=False,
                   num_devices=N_CORES)

    # ---- DRAM parameters (per-core data via in_maps) ----
    embT_ext = nc.declare_dram_parameter("embT", [128, KC_D, ROWS], F32, isOutput=False)
    wb_shapes = dict(Wgt=(D, D), V0=(D, H), V1=(H, D), W1=(D, D),
                     W2Wg=(D, D), W2=(D, D), R=(D, D), RWt=(D, D))
    wb_ext = {}
    for name, (wk, wm) in wb_shapes.items():
        wb_ext[name] = nc.declare_dram_parameter(f"wb_{name}", [128, wk // 128, wm],
                                                 BF16, isOutput=False)
    sel_ext = nc.declare_dram_parameter("sel", [128, 8], F32, isOutput=False)
    csn_ext = nc.declare_dram_parameter("csn", [128, KC_D, 2], F32, isOutput=False)
    wl_ext = nc.declare_dram_parameter("wl", [NV, 128, KC_D, 128], BF16, isOutput=False)
    out_ext = nc.declare_dram_parameter("out", [2, VPAD, N_CORES * HALF], F32,
                                        isOutput=True)
    warm_ext = nc.declare_dram_parameter("warm", [128, 4], F32, isOutput=True)

    # ---- internal DRAM for collectives ----
    halo_in = [nc.dram_tensor(f"halo_in_{k}", [128, KC_D * 8], F32)
               for k in range(n_iters)]
    halo_out = [nc.dram_tensor(f"halo_out_{k}", [N_CORES * 128, KC_D * 8], F32,
                               addr_space="Shared") for k in range(n_iters)]
    ccw_in = nc.dram_tensor("ccw_in", [1, 32], F32)
    ccw_out = nc.dram_tensor("ccw_out", [N_CORES, 32], F32, addr_space="Shared")
    hfa_in = nc.dram_tensor("hfa_in", [128, KC_D * HALF], BF16)
    hfa_out = nc.dram_tensor("hfa_out", [N_CORES * 128, KC_D * HALF], BF16,
                             addr_space="Shared")
    hfb_in = nc.dram_tensor("hfb_in", [128, KC_D * HALF], BF16)
    hfb_out = nc.dram_tensor("hfb_out", [N_CORES * 128, KC_D * HALF], BF16,
                             addr_space="Shared")

    rg = [list(range(N_CORES))]

    with tile.TileContext(nc) as tc, (
            tc.tile_pool(name="cpool", bufs=1)) as cpool, (
            tc.tile_pool(name="apool", bufs=1)) as apool, (
            tc.tile_pool(name="npool", bufs=1)) as npool:
        with (
            tc.tile_pool(name="wpool", bufs=1) as wpool,      # small weights
            tc.tile_pool(name="wvpool", bufs=1) as wvpool,    # V0/V1 (freed early)
            tc.tile_pool(name="pps", bufs=2, space="PSUM") as pps,
            tc.tile_pool(name="rqps", bufs=4, space="PSUM") as rqps,
            tc.tile_pool(name="sps", bufs=2, space="PSUM") as sps,
        ):
            # ---------- load persistent data (order = DMA priority) ----------
            embT = cpool.tile([128, KC_D, ROWS], F32, tag="embT")
            nc.sync.dma_start(embT[:], embT_ext[:])
            wsb = {}
            for name in ("Wgt", "V0", "V1", "W1", "W2Wg", "W2", "R", "RWt"):
                ext = wb_ext[name]
                pool = wvpool if name in ("V0", "V1") else wpool
                t_ = pool.tile(list(ext.shape), BF16, tag=f"w_{name}",
                               name=f"w_{name}")
                nc.sync.dma_start(t_[:], ext[:])
                wsb[name] = t_
            sel = cpool.tile([128, 8], F32, tag="sel")
            nc.sync.dma_start(sel[:], sel_ext[:])
            csn = cpool.tile([128, KC_D, 2], F32, tag="csn")
            nc.sync.dma_start(csn[:], csn_ext[:])
            embTbf = cpool.tile([128, KC_D, ROWS], BF16, tag="embTbf")
            nc.vector.tensor_copy(embTbf[:], embT[:])
            # warm up the collective path early (first call pays ENCD init)
            nc.sync.dma_start(ccw_in[:], embT[0:1, 0, 0:32])
            nc.gpsimd.collective_compute(
                "AllGather", ALU.bypass, replica_groups=rg,
                ins=[ccw_in[:]], outs=[ccw_out[:]])

            ones_col_bf = cpool.tile([128, 1], BF16, tag="ones_col_bf")
            nc.vector.memset(ones_col_bf[:], 1.0)
            ones_row_f = cpool.tile([1, 128], F32, tag="ones_row_f")
            nc.vector.memset(ones_row_f[:], 1.0)

            # PE warm-up on the first-landing weight (HAM un-throttle).
            wu_ps = pps.tile([128, 512], F32, tag="mmps")
            for i in range(12):
                nc.tensor.matmul(wu_ps[:], wsb["Wgt"][:, 0, 0:128],
                                 wsb["Wgt"][:, 1, 0:512], start=True, stop=True)
            wu_sb = cpool.tile([128, 4], F32, tag="wu_sb")
            nc.vector.tensor_copy(wu_sb[:], wu_ps[:, 0:4])
            nc.sync.dma_start(warm_ext[:], wu_sb[:])

            # helper: one weight "layer": out chunks [mc] = sum_kc lhsT @ rhs
            def mm_layer(wname, Kc, Mc, rhs_fn, consume, group=4, mid=None):
                """consume(psum_ap, m0, g) handles [128, g*128] out; mid() is
                emitted after the first group's matmuls (PE-queue slot for
                broadcast matmuls whose inputs are still being computed)."""
                w = wsb[wname]
                for gi, m0 in enumerate(range(0, Mc, group)):
                    g = min(group, Mc - m0)
                    p = pps.tile([128, g * 128], F32, tag="mmps")
                    for sub in range(g):
                        mc = m0 + sub
                        for kc in range(Kc):
                            nc.tensor.matmul(
                                p[:, sub * 128:(sub + 1) * 128],
                                w[:, kc, mc * 128:(mc + 1) * 128],
                                rhs_fn(kc),
                                start=(kc == 0), stop=(kc == Kc - 1))
                    if gi == 0 and mid is not None:
                        mid()
                    consume(p, m0, g)

            embT_ = embT
            TILE_NAMES.clear()

            # ---------- precompute EG = embT @ Wg_top ----------
            EG = cpool.tile([128, KC_D, ROWS], F32, tag="EG")

            def eg_consume(p, m0, g):
                nc.vector.tensor_copy(EG[:, m0:m0 + g, :], p[:])
            mm_layer("Wgt", KC_D, KC_D, lambda kc: embTbf[:, kc, :], eg_consume)

            # ---------- Picard sweeps ----------
            CTXs = [cpool.tile([128, KC_D, ROWS], BF16, tag=f"CTX{i}",
                               name=f"CTX{i}") for i in range(2)]
            TFs = [cpool.tile([128, KC_D, ROWS], F32, tag=f"TF{i}",
                              name=f"TF{i}") for i in range(2)]
            Hbf = cpool.tile([128, KC_D, ROWS], BF16, tag="Hbf")

            for it in range(n_iters):
                first = (it == 0)
                last = (it == n_iters - 1)
                CTX = embTbf if first else CTXs[it % 2]
                TFc = None if first else TFs[it % 2]
                CTXn = CTXs[(it + 1) % 2]
                TFn = TFs[(it + 1) % 2]

                # A = gelu(ctx@V0)  (single ScalarE op per group, PSUM -> SBUF)
                Abf = apool.tile([128, KC_H, ROWS], BF16, tag="Abf")

                def a_consume(p, m0, g):
                    nc.scalar.activation(Abf[:, m0:m0 + g, :], p[:], AF.Gelu)
                mm_layer("V0", KC_D, KC_H, lambda kc: CTX[:, kc, :], a_consume)

                # TGT = gelu(A@V1); also accumulate sum(TGT^2) rows
                TGTbf = apool.tile([128, KC_D, ROWS], BF16, tag="TGTbf", bufs=2)
                ssp = sps.tile([1, 2 * ROWS], F32, tag="sums")
                sq = npool.tile([128, KC_D, ROWS], BF16, tag="sqh", bufs=2)

                def t_consume(p, m0, g):
                    nc.scalar.activation(TGTbf[:, m0:m0 + g, :], p[:], AF.Gelu)
                    nc.vector.tensor_tensor(
                        sq[:, m0:m0 + g, :], TGTbf[:, m0:m0 + g, :],
                        TGTbf[:, m0:m0 + g, :], ALU.mult)
                    for sub in range(g):
                        mc = m0 + sub
                        nc.tensor.matmul(ssp[:, 0:ROWS], ones_col_bf[:],
                                         sq[:, mc, :],
                                         start=(mc == 0), stop=(mc == KC_D - 1))
                mm_layer("V1", KC_H, KC_D, lambda kc: Abf[:, kc, :], t_consume)

                # boundary rows 0:4 of CTXn/TFn for sweep it+1 (early, off the
                # critical tail; the lag-2 halo AG has long completed)
                if not last:
                    if it >= 1:
                        blocks = npool.tile([128, 8, KC_D * 8], F32, tag="blocks")
                        nc.sync.dma_start(
                            blocks[:],
                            halo_out[it - 1].ap().rearrange("(r p) f -> p r f",
                                                            p=128))
                        hacc = npool.tile([128, KC_D * 8], F32, tag="hacc")
                        nc.vector.tensor_scalar(hacc[:], blocks[:, 0, :],
                                                sel[:, 0:1], None, ALU.mult)
                        for r in range(1, N_CORES):
                            nc.vector.scalar_tensor_tensor(
                                hacc[:], blocks[:, r, :], sel[:, r:r + 1],
                                hacc[:], ALU.mult, ALU.add)
                        ha = hacc[:].rearrange("p (k c) -> p k c", k=KC_D)
                        for kc in range(KC_D):
                            nc.vector.tensor_tensor(
                                CTXn[:, kc, 0:4], ha[:, kc, 0:4],
                                embT_[:, kc, 0:4], ALU.add)
                        nc.vector.tensor_copy(TFn[:, :, 0:4], ha[:, :, 4:8])
                    else:
                        for kc in range(KC_D):
                            nc.vector.tensor_copy(CTXn[:, kc, 0:4],
                                                  embT_[:, kc, 0:4])
                        nc.vector.memset(TFn[:, :, 0:4], 0.0)

                # l2n scale s on DVE (runs under W1's first matmul group)
                ss = npool.tile([1, ROWS], F32, tag="ss")
                nc.vector.tensor_scalar(ss[:], ssp[:, 0:ROWS], 1e-24, None,
                                        ALU.add)
                r_l2 = _nr_rsqrt(nc, npool, ss[:], ROWS, "l2n", iters=2)
                rb = npool.tile([128, ROWS], F32, tag="rb")

                def w1_mid():
                    rb_p = pps.tile([128, ROWS], F32, tag="mmps")
                    nc.tensor.matmul(rb_p[:], ones_row_f[:], r_l2[:],
                                     start=True, stop=True)
                    nc.vector.tensor_copy(rb[:], rb_p[:])

                # U = gelu(s*(TGT@W1))
                Ubf = apool.tile([128, KC_D, ROWS], BF16, tag="Ubf", bufs=2)

                def u_consume(p, m0, g):
                    ysc = apool.tile([128, g * 128], F32, tag=f"ysc{m0 % 8}")
                    for sub in range(g):
                        nc.vector.tensor_tensor(
                            ysc[:, sub * 128:(sub + 1) * 128],
                            p[:, sub * 128:(sub + 1) * 128], rb[:], ALU.mult)
                    nc.scalar.activation(Ubf[:, m0:m0 + g, :], ysc[:], AF.Gelu)
                mm_layer("W1", KC_D, KC_D, lambda kc: TGTbf[:, kc, :], u_consume,
                         mid=w1_mid)

                # G = sigmoid(EG + U@(W2@Wg_bot)) = 0.5 + 0.5*tanh(x/2)
                Gsb = apool.tile([128, KC_D, ROWS], F32, tag="Gsb")

                def g_consume(p, m0, g):
                    gin = apool.tile([128, g * 128], F32, tag=f"sc{m0 % 8}")
                    nc.vector.tensor_tensor(gin[:], p[:], EG[:, m0:m0 + g, :],
                                            ALU.add)
                    th = apool.tile([128, g * 128], F32, tag=f"th{m0 % 8}")
                    nc.scalar.activation(th[:], gin[:], AF.Tanh, scale=0.5)
                    nc.vector.tensor_scalar(Gsb[:, m0:m0 + g, :], th[:], 0.5,
                                            0.5, ALU.mult, ALU.add)
                mm_layer("W2Wg", KC_D, KC_D, lambda kc: Ubf[:, kc, :], g_consume)

                # CF = U@W2; hpre = G*(CF + TFc - EMB) + EMB (TFc pre-alpha'd)
                hpre = apool.tile([128, KC_D, ROWS], F32, tag="hpre")
                hpre_bf = npool.tile([128, KC_D, ROWS], BF16, tag="hpre_bf",
                                     bufs=2)
                hsq = npool.tile([128, KC_D, ROWS], BF16, tag="sqh", bufs=2)

                def hp_consume(p, m0, g):
                    t1 = apool.tile([128, g * 128], F32, tag=f"t1_{m0 % 8}")
                    if first:
                        nc.vector.tensor_tensor(
                            t1[:], p[:], embT_[:, m0:m0 + g, :], ALU.subtract)
                    else:
                        nc.vector.tensor_tensor(
                            t1[:], p[:], TFc[:, m0:m0 + g, :], ALU.add)
                        nc.vector.tensor_tensor(
                            t1[:], t1[:], embT_[:, m0:m0 + g, :], ALU.subtract)
                    nc.vector.tensor_tensor(t1[:], t1[:], Gsb[:, m0:m0 + g, :],
                                            ALU.mult)
                    nc.vector.tensor_tensor(hpre[:, m0:m0 + g, :], t1[:],
                                            embT_[:, m0:m0 + g, :], ALU.add)
                    nc.vector.tensor_copy(hpre_bf[:, m0:m0 + g, :],
                                          hpre[:, m0:m0 + g, :])
                    nc.vector.tensor_tensor(hsq[:, m0:m0 + g, :],
                                            hpre[:, m0:m0 + g, :],
                                            hpre[:, m0:m0 + g, :], ALU.mult)
                mm_layer("W2", KC_D, KC_D, lambda kc: Ubf[:, kc, :], hp_consume)

                # ---- sweep tail: R/RWt on pre-LN hpre + LN stats, all PE
                # work ordered so nothing in the in-order queue waits ----
                s12 = sps.tile([1, 2 * ROWS], F32, tag="sums")

                def stat_mms():
                    for mc in range(KC_D):
                        nc.tensor.matmul(s12[:, 0:ROWS], ones_col_bf[:],
                                         hpre_bf[:, mc, :],
                                         start=(mc == 0), stop=(mc == KC_D - 1))
                    for mc in range(KC_D):
                        nc.tensor.matmul(s12[:, ROWS:2 * ROWS], ones_col_bf[:],
                                         hsq[:, mc, :],
                                         start=(mc == 0), stop=(mc == KC_D - 1))

                def stats_chain():
                    mrow = npool.tile([1, ROWS], F32, tag="mrow")
                    nc.vector.tensor_scalar(mrow[:], s12[:, 0:ROWS], 1.0 / D,
                                            None, ALU.mult)
                    var = npool.tile([1, ROWS], F32, tag="var")
                    nc.vector.tensor_tensor(var[:], mrow[:], mrow[:], ALU.mult)
                    nc.vector.scalar_tensor_tensor(
                        var[:], s12[:, ROWS:2 * ROWS], 1.0 / D, var[:],
                        ALU.mult, ALU.subtract)
                    nc.vector.tensor_scalar(var[:], var[:], 1e-5, None, ALU.add)
                    r_ln = _nr_rsqrt(nc, npool, var[:], ROWS, "ln", iters=2)
                    return mrow, r_ln

                if last:
                    stat_mms()
                    mrow, r_ln = stats_chain()
                    mb_p = pps.tile([128, ROWS], F32, tag="mmps")
                    nc.tensor.matmul(mb_p[:], ones_row_f[:], mrow[:],
                                     start=True, stop=True)
                    rb2_p = pps.tile([128, ROWS], F32, tag="mmps")
                    nc.tensor.matmul(rb2_p[:], ones_row_f[:], r_ln[:],
                                     start=True, stop=True)
                    for kc in range(KC_D):
                        d_ = npool.tile([128, ROWS], F32, tag=f"lnd{kc % 3}",
                                        name=f"lnd{kc}")
                        nc.vector.tensor_tensor(d_[:], hpre[:, kc, :], mb_p[:],
                                                ALU.subtract)
                        nc.vector.tensor_tensor(Hbf[:, kc, :], d_[:], rb2_p[:],
                                                ALU.mult)
                    continue

                rq_list = []

                def rq_mms(wname, m0, g):
                    p = rqps.tile([128, g * 128], F32, tag="rqp")
                    for sub in range(g):
                        mc = m0 + sub
                        for kc in range(KC_D):
                            nc.tensor.matmul(
                                p[:, sub * 128:(sub + 1) * 128],
                                wsb[wname][:, kc, mc * 128:(mc + 1) * 128],
                                hpre_bf[:, kc, :],
                                start=(kc == 0), stop=(kc == KC_D - 1))
                    rq_list.append((p, m0, g))

                rq_mms("R", 0, 3)
                stat_mms()
                rq_mms("R", 3, 3)
                rq_mms("RWt", 0, 3)
                mrow, r_ln = stats_chain()
                as_row = npool.tile([1, ROWS], F32, tag="as_row")
                nc.vector.tensor_scalar(as_row[:], r_ln[:], ALPHA, None,
                                        ALU.mult)
                # broadcasts sit in the PE queue behind RWt g0: stats are
                # ready by then, so they never block the stream
                mb_p = pps.tile([128, ROWS], F32, tag="mmps")
                nc.tensor.matmul(mb_p[:], ones_row_f[:], mrow[:], start=True,
                                 stop=True)
                asb_p = pps.tile([128, ROWS], F32, tag="mmps")
                nc.tensor.matmul(asb_p[:], ones_row_f[:], as_row[:], start=True,
                                 stop=True)
                mb = npool.tile([128, ROWS], F32, tag="mb")
                nc.vector.tensor_copy(mb[:], mb_p[:])
                asb = npool.tile([128, ROWS], F32, tag="asb")
                nc.vector.tensor_copy(asb[:], asb_p[:])
                rq_mms("RWt", 3, 3)

                # deferred consumes: CTXn/TFn interiors + halo edges
                Ppre = apool.tile([128, KC_D, ROWS], F32, tag="Ppre")
                hQe = apool.tile([128, KC_D, 4], F32, tag="hQe", bufs=2)
                for gi, (p, m0, g) in enumerate(rq_list):
                    is_r = gi < 2
                    for sub in range(g):
                        mc = m0 + sub
                        sl = slice(sub * 128, (sub + 1) * 128)
                        tmp = apool.tile([128, ROWS], F32, tag=f"rtmp{mc % 4}")
                        nc.vector.scalar_tensor_tensor(
                            tmp[:], mb[:], csn[:, mc, 0:1] if is_r
                            else csn[:, mc, 1:2], p[:, sl], ALU.mult, ALU.add)
                        if is_r:
                            nc.vector.tensor_tensor(Ppre[:, mc, :], tmp[:],
                                                    asb[:], ALU.mult)
                            nc.vector.tensor_tensor(
                                CTXn[:, mc, 4:ROWS], Ppre[:, mc, 0:ROWS - 4],
                                embT_[:, mc, 4:ROWS], ALU.add)
                        else:
                            nc.vector.tensor_tensor(
                                TFn[:, mc, 4:ROWS], tmp[:, 0:ROWS - 4],
                                asb[:, 0:ROWS - 4], ALU.mult)
                            nc.vector.tensor_tensor(
                                hQe[:, mc, :], tmp[:, ROWS - 4:ROWS],
                                asb[:, ROWS - 4:ROWS], ALU.mult)

                # launch my halo (P edge + Q edge) for sweep it+2
                if it + 2 < n_iters:
                    hio = halo_in[it].ap().rearrange("p (k c) -> p k c", k=KC_D)
                    nc.sync.dma_start(hio[:, :, 0:4], Ppre[:, :, ROWS - 4:ROWS])
                    nc.sync.dma_start(hio[:, :, 4:8], hQe[:])
                    nc.gpsimd.collective_compute(
                        "AllGather", ALU.bypass, replica_groups=rg,
                        ins=[halo_in[it][:]], outs=[halo_out[it][:]])

            # ---------- final 2-chunk AllGather of Hbf ----------
            nc.sync.dma_start(hfa_in.ap().rearrange("p (k c) -> p k c", k=KC_D),
                              Hbf[:, :, 0:HALF])
            nc.gpsimd.collective_compute(
                "AllGather", ALU.bypass, replica_groups=rg,
                ins=[hfa_in[:]], outs=[hfa_out[:]])
            nc.sync.dma_start(hfb_in.ap().rearrange("p (k c) -> p k c", k=KC_D),
                              Hbf[:, :, HALF:ROWS])
            nc.gpsimd.collective_compute(
                "AllGather", ALU.bypass, replica_groups=rg,
                ins=[hfb_in[:]], outs=[hfb_out[:]])

        # ---------- lm_head: logits^T = Wl^T @ H^T, vocab-sharded, bf16.
        # Wl is streamed (the pool allocates into V0/V1's freed zone, so the
        # stream starts during the final sweep); the H AllGather is split in
        # two row-halves so chunk b overlaps phase-a matmuls. ----------
        with (
            tc.tile_pool(name="lmpool", bufs=8) as lmpool,
            tc.tile_pool(name="opool", bufs=4) as opool,
            tc.tile_pool(name="lps", bufs=6, space="PSUM") as lps,
        ):
            Hra = lmpool.tile([128, KC_D, N_CORES * HALF], BF16, tag="Hra",
                              bufs=1)
            nc.sync.dma_start(
                Hra[:].rearrange("p k (r c) -> p k r c", r=N_CORES),
                hfa_out.ap().rearrange("(r p) (k c) -> p k r c", p=128, k=KC_D))
            Hrb = lmpool.tile([128, KC_D, N_CORES * HALF], BF16, tag="Hrb",
                              bufs=1)
            nc.sync.dma_start(
                Hrb[:].rearrange("p k (r c) -> p k r c", r=N_CORES),
                hfb_out.ap().rearrange("(r p) (k c) -> p k r c", p=128, k=KC_D))

            for half, Hr in ((0, Hra), (1, Hrb)):
                for vc in range(NV):
                    wl_t = lmpool.tile([128, KC_D, 128], BF16, tag="wl")
                    nc.sync.dma_start(wl_t[:], wl_ext[vc])
                    p = lps.tile([128, N_CORES * HALF], F32, tag="lmp")
                    for kc in range(KC_D):
                        nc.tensor.matmul(
                            p[:], wl_t[:, kc, :], Hr[:, kc, :],
                            start=(kc == 0), stop=(kc == KC_D - 1))
                    osb = opool.tile([128, N_CORES * HALF], F32, tag="osb")
                    if vc % 2 == 0:
                        nc.vector.tensor_copy(osb[:], p[:])
                    else:
                        nc.scalar.copy(osb[:], p[:])
                    nc.sync.dma_start(
                        out_ext[half, vc * 128:(vc + 1) * 128, :], osb[:])

    nc.compile()
    return nc


def _get_built(n_iters=N_ITERS):
    if n_iters not in _BUILD_CACHE:
        _BUILD_CACHE[n_iters] = build(n_iters)
    return _BUILD_CACHE[n_iters]


def _prep_in_maps(token_ids, embedding, V0, b0, V1, b1, W1, c1, W2, c2, Wg, bg,
                  Wt, gamma, beta, Wl, R_weight):
    f64 = np.float64
    for z in (b0, b1, c1, c2, bg, beta):
        assert np.count_nonzero(np.asarray(z)) == 0, "nonzero bias unsupported"
    assert np.allclose(np.asarray(gamma), 1.0), "gamma != 1 unsupported"

    tok = np.asarray(token_ids).astype(np.int64)           # [B, T]
    emb = np.asarray(embedding, f64)[tok]                  # [B, T, D]
    emb = emb / np.maximum(np.linalg.norm(emb, axis=-1, keepdims=True), 1e-12)
    rows = emb.transpose(1, 0, 2).reshape(T * B, D)        # row = t*4+b

    bf = ml_dtypes.bfloat16
    R_bf = _t_layout(np.asarray(R_weight, f64)).astype(bf)
    RWt_bf = _t_layout(np.asarray(R_weight, f64) @ np.asarray(Wt, f64)).astype(bf)
    wt = {
        "R": R_bf,
        "V0": _t_layout(np.asarray(V0, f64)).astype(bf),
        "V1": _t_layout(np.asarray(V1, f64)).astype(bf),
        "W1": _t_layout(np.asarray(W1, f64)).astype(bf),
        "W2": _t_layout(np.asarray(W2, f64)).astype(bf),
        "RWt": RWt_bf,
        "Wgt": _t_layout(np.asarray(Wg, f64)[:D]).astype(bf),
        "W2Wg": _t_layout(np.asarray(W2, f64) @ np.asarray(Wg, f64)[D:]).astype(bf),
    }
    # negated col-sums of the bf16-rounded R / RWt, in T-layout [128, KC_D]
    csn = np.zeros((128, KC_D, 2), np.float32)
    for i, w_bf in enumerate((R_bf, RWt_bf)):
        cs = -w_bf.astype(f64).sum(axis=(0, 1))            # [-colsum] over K
        csn[:, :, i] = cs.reshape(KC_D, 128).T
    wl_bf = np.asarray(Wl, f64).astype(bf)

    in_maps = []
    for c in range(N_CORES):
        block = rows[c * ROWS:(c + 1) * ROWS].T            # [D, 128]
        embT = np.ascontiguousarray(
            block.reshape(KC_D, 128, ROWS).transpose(1, 0, 2)).astype(np.float32)
        sel = np.zeros((128, 8), np.float32)
        if c > 0:
            sel[:, c - 1] = 1.0
        wl_shard_cols = np.zeros((D, VPAD), bf)
        lo = c * VSHARD
        hi = min(V, lo + VSHARD)
        wl_shard_cols[:, :hi - lo] = wl_bf[:, lo:hi]
        wl_shard = _t_layout(wl_shard_cols)                 # [128, KC_D, VPAD]
        wl_shard = np.ascontiguousarray(
            wl_shard.reshape(128, KC_D, NV, 128).transpose(2, 0, 1, 3))
        m = {"embT": embT, "sel": sel, "csn": csn, "wl": wl_shard}
        for name, w in wt.items():
            m[f"wb_{name}"] = w
        in_maps.append(m)
    return in_maps


def kernel(**inputs):
    global LAST_RESULT
    in_maps = _prep_in_maps(**{k: np.asarray(v) for k, v in inputs.items()})
    nc = _get_built()
    trace = bool(os.environ.get("KERNEL_TRACE"))
    res = run_bass_kernel_spmd(nc, in_maps, core_ids=list(range(N_CORES)),
                               trace=trace)
    LAST_RESULT = res
    parts = []
    for c in range(N_CORES):
        o = res.results[c]["out"]                          # [2, VPAD, 8*HALF]
        o = o.reshape(2, VPAD, N_CORES, HALF)
        o = o.transpose(1, 2, 0, 3).reshape(VPAD, T * B)   # row = core*128+half*64+i
        parts.append(o[:VSHARD])
    L = np.concatenate(parts, axis=0)[:V]                  # [V, T*B]
    out = np.ascontiguousarray(
        L.reshape(V, T, B).transpose(2, 1, 0)).astype(np.float32)
    return out


if __name__ == "__main__":
    pass


# revision 23
# speedup vs baseline: 1.1840x; 1.1087x over previous
"""Trainium2 Bass kernel for nn_AgnisV5 (B=4, T=256, V=50257, D=768, H=3072).

Strategy
--------
The reference is a 256-step sequential recurrence over h (LayerNorm'd each
step) plus a big lm_head projection that does not feed back. The recurrence
map is contractive (Jacobian norm ~0.65), so instead of stepping 256 times
with tiny (M=4) matmuls, we solve the whole sequence by 12 batched Picard
sweeps: H <- StepAll(shift(H)), each sweep a full-width (M=128/core) pass
over all timesteps. bf16 sweeps floor at ~2.8e-3 of logits scale; 12 sweeps
measure ~1.2e-2 of scale (gate 2e-2).

Sharding: time-sharded across 8 cores (128 rows = 32 timesteps x batch 4 per
core), weights replicated in bf16 SBUF-resident form. Cross-core traffic is a
tiny per-sweep boundary halo (lag-2, fully overlapped AllGather) plus a final
2-chunk AllGather of H (bf16) for the vocab-sharded bf16 lm_head, overlapped
with the first lm_head phase.

Keeping the PE dense (v3):
  - All gelus via the exact-gelu ACT table; sigmoid(x) = 0.5+0.5*tanh(x/2)
    keeps the gate in the same table set (no ACT table switches). Consumes
    become a single ScalarE op reading PSUM directly.
  - l2n fold: U = gelu(l2n(TGT)@W1) = gelu(s*(TGT@W1)) -- W1 runs on raw
    TGT while the norm chain computes s on DVE; the s-broadcast matmul is
    placed mid-W1 so it never stalls the in-order PE queue.
  - LN fold: LN(hpre)@R = s*(hpre@R - m*colsum(R)) -- next sweep's R/RWt
    matmuls run on pre-LN hpre; LN stats apply as a DVE correction with
    deferred psum consumes, stat/broadcast matmuls slotted between R/RWt
    groups. The halo ships pre-transformed alpha*s*(P - m*csR) edges.
  - V0/V1 live in their own tile pool so their SBUF frees at last use,
    letting the lm_head's Wl stream start during the final sweep.
  - rsqrt via DVE Newton iterations (no ACT table switch).
"""
import sys, os
sys.path.insert(0, '/opt/trn_rl_repo')
import numpy as np
import ml_dtypes

import concourse.bass as bass
import concourse.bacc as bacc
import concourse.mybir as mybir
import concourse.tile as tile
from concourse.bass_utils import run_bass_kernel_spmd


def _ensure_ntff_hook():
    """The agent image's antenv lacks axon_hooks, which silently disables
    NTFF profiling (exec_time_ns). Shim the module and register the
    ctypes-based hook from trn_agent_boot if available."""
    import types
    if "antenv.axon_hooks" in sys.modules:
        return
    try:
        import antenv
        m = types.ModuleType("antenv.axon_hooks")
        _h = [None]
        m.set_axon_ntff_profile_hook = lambda h: _h.__setitem__(0, h)
        m.get_axon_ntff_profile_hook = lambda: _h[0]
        sys.modules["antenv.axon_hooks"] = m
        antenv.axon_hooks = m
        from trn_agent_boot.trn_boot import _ntff_profile_via_ctypes
        hook = _ntff_profile_via_ctypes("/opt/axon/libaxon_pjrt.so")
        if hook is not None:
            m.set_axon_ntff_profile_hook(hook)
    except Exception:
        pass


_ensure_ntff_hook()

F32 = mybir.dt.float32
BF16 = mybir.dt.bfloat16
AF = mybir.ActivationFunctionType
ALU = mybir.AluOpType

N_CORES = 8
B, T, V, D, H = 4, 256, 50257, 768, 3072
ROWS = 128                 # rows per core = 32 timesteps x 4 batch
KC_D = D // 128            # 6 chunks of the d dimension
KC_H = H // 128            # 24 chunks of the hidden dimension
VPAD = 6400                # per-core vocab shard cols, padded to 50*128
VSHARD = 6283              # ceil(V / 8); host pads vocab to 8*VSHARD = 50264
NV = VPAD // 128           # 50 vocab chunks per core
HALF = ROWS // 2           # final AllGather row-chunk size
N_ITERS = 12
ALPHA = 0.4

LAST_RESULT = None         # BassKernelResults of the most recent run (for test.py)
TILE_NAMES = {}

_BUILD_CACHE = {}


def _t_layout(w):
    """[K, M] row-major -> [128, K/128, M] T-layout for stationary lhsT tiles."""
    K, M = w.shape
    assert K % 128 == 0
    return np.ascontiguousarray(w.reshape(K // 128, 128, M).transpose(1, 0, 2))


def _nr_rsqrt(nc, pool, s_ap, n_free, name, iters=1):
    """rsqrt(s) on DVE: bit-trick seed + Newton iterations. s_ap: [1, n] f32."""
    bits = pool.tile([1, n_free], mybir.dt.int32, tag=f"{name}_bits")
    nc.vector.tensor_scalar(bits[:], s_ap.bitcast(mybir.dt.int32), 1, None,
                            ALU.logical_shift_right)
    nc.vector.tensor_scalar(bits[:], bits[:], -1, 0x5f3759df, ALU.mult, ALU.add)
    y = pool.tile([1, n_free], F32, tag=f"{name}_y")
    nc.vector.tensor_copy(y[:], bits[:].bitcast(F32))
    half = pool.tile([1, n_free], F32, tag=f"{name}_half")
    nc.vector.tensor_scalar(half[:], s_ap, 0.5, None, ALU.mult)
    yy = pool.tile([1, n_free], F32, tag=f"{name}_yy")
    e = pool.tile([1, n_free], F32, tag=f"{name}_e")
    for _ in range(iters):
        nc.vector.tensor_tensor(yy[:], y[:], y[:], ALU.mult)
        nc.vector.tensor_tensor(e[:], yy[:], half[:], ALU.mult)
        nc.vector.tensor_scalar(e[:], e[:], -1.0, 1.5, ALU.mult, ALU.add)
        nc.vector.tensor_tensor(y[:], y[:], e[:], ALU.mult)
    return y


def build(n_iters=N_ITERS):
    nc = bacc.Bacc("TRN2", target_bir_lowering=False, debug=False,
                   num_devices=N_CORES)

    # ---- DRAM parameters (per-core data via in_maps) ----
    embT_ext = nc.declare_dram_parameter("embT", [128, KC_D, ROWS], F32, isOutput=False)
    wb_shapes = dict(Wgt=(D, D), V0=(D, H), V1=(H, D), W1=(D, D),
                     W2Wg=(D, D), W2=(D, D), R=(D, D), RWt=(D, D))
    wb_ext = {}
    for name, (wk, wm) in wb_shapes.items():
        wb_ext[name] = nc.declare_dram_parameter(f"wb_{name}", [128, wk // 128, wm],
                                                 BF16, isOutput=False)
    sel_ext = nc.declare_dram_parameter("sel", [128, 8], F32, isOutput=False)
    csn_ext = nc.declare_dram_parameter("csn", [128, KC_D, 2], F32, isOutput=False)
    wl_ext = nc.declare_dram_parameter("wl", [NV, 128, KC_D, 128], BF16, isOutput=False)
    out_ext = nc.declare_dram_parameter("out", [2, VPAD, N_CORES * HALF], F32,
                                        isOutput=True)
    warm_ext = nc.declare_dram_parameter("warm", [128, 4], F32, isOutput=True)

    # ---- internal DRAM for collectives ----
    halo_in = [nc.dram_tensor(f"halo_in_{k}", [128, KC_D * 8], F32)
               for k in range(n_iters)]
    halo_out = [nc.dram_tensor(f"halo_out_{k}", [N_CORES * 128, KC_D * 8], F32,
                               addr_space="Shared") for k in range(n_iters)]
    ccw_in = nc.dram_tensor("ccw_in", [1, 32], F32)
    ccw_out = nc.dram_tensor("ccw_out", [N_CORES, 32], F32, addr_space="Shared")
    hfa_in = nc.dram_tensor("hfa_in", [128, KC_D * HALF], BF16)
    hfa_out = nc.dram_tensor("hfa_out", [N_CORES * 128, KC_D * HALF], BF16,
                             addr_space="Shared")
    hfb_in = nc.dram_tensor("hfb_in", [128, KC_D * HALF], BF16)
    hfb_out = nc.dram_tensor("hfb_out", [N_CORES * 128, KC_D * HALF], BF16,
                             addr_space="Shared")

    rg = [list(range(N_CORES))]

    with tile.TileContext(nc) as tc, (
            tc.tile_pool(name="cpool", bufs=1)) as cpool, (
            tc.tile_pool(name="apool", bufs=1)) as apool, (
            tc.tile_pool(name="npool", bufs=1)) as npool:
        with (
            tc.tile_pool(name="wpool", bufs=1) as wpool,      # small weights
            tc.tile_pool(name="wvpool", bufs=1) as wvpool,    # V0/V1 (freed early)
            tc.tile_pool(name="pps", bufs=2, space="PSUM") as pps,
            tc.tile_pool(name="rqps", bufs=4, space="PSUM") as rqps,
            tc.tile_pool(name="sps", bufs=2, space="PSUM") as sps,
        ):
            # ---------- load persistent data (order = DMA priority) ----------
            embT = cpool.tile([128, KC_D, ROWS], F32, tag="embT")
            nc.sync.dma_start(embT[:], embT_ext[:])
            wsb = {}
            for name in ("Wgt", "V0", "V1", "W1", "W2Wg", "W2", "R", "RWt"):
                ext = wb_ext[name]
                pool = wvpool if name in ("V0", "V1") else wpool
                t_ = pool.tile(list(ext.shape), BF16, tag=f"w_{name}",
                               name=f"w_{name}")
                nc.sync.dma_start(t_[:], ext[:])
                wsb[name] = t_
            sel = cpool.tile([128, 8], F32, tag="sel")
            nc.sync.dma_start(sel[:], sel_ext[:])
            csn = cpool.tile([128, KC_D, 2], F32, tag="csn")
            nc.sync.dma_start(csn[:], csn_ext[:])
            embTbf = cpool.tile([128, KC_D, ROWS], BF16, tag="embTbf")
            nc.vector.tensor_copy(embTbf[:], embT[:])
            # warm up the collective path early (first call pays ENCD init)
            nc.sync.dma_start(ccw_in[:], embT[0:1, 0, 0:32])
            nc.gpsimd.collective_compute(
                "AllGather", ALU.bypass, replica_groups=rg,
                ins=[ccw_in[:]], outs=[ccw_out[:]])

            ones_col_bf = cpool.tile([128, 1], BF16, tag="ones_col_bf")
            nc.vector.memset(ones_col_bf[:], 1.0)
            ones_row_bf = cpool.tile([1, 128], BF16, tag="ones_row_bf")
            nc.vector.memset(ones_row_bf[:], 1.0)

            # PE warm-up on the first-landing weight (HAM un-throttle).
            wu_ps = pps.tile([128, 512], F32, tag="mmps")
            for i in range(12):
                nc.tensor.matmul(wu_ps[:], wsb["Wgt"][:, 0, 0:128],
                                 wsb["Wgt"][:, 1, 0:512], start=True, stop=True)
            wu_sb = cpool.tile([128, 4], F32, tag="wu_sb")
            nc.vector.tensor_copy(wu_sb[:], wu_ps[:, 0:4])
            nc.sync.dma_start(warm_ext[:], wu_sb[:])

            # helper: one weight "layer": out chunks [mc] = sum_kc lhsT @ rhs
            def mm_layer(wname, Kc, Mc, rhs_fn, consume, group=4, mid=None):
                """consume(psum_ap, m0, g) handles [128, g*128] out; mid() is
                emitted after the first group's matmuls (PE-queue slot for
                broadcast matmuls whose inputs are still being computed)."""
                w = wsb[wname]
                for gi, m0 in enumerate(range(0, Mc, group)):
                    g = min(group, Mc - m0)
                    p = pps.tile([128, g * 128], F32, tag="mmps")
                    for sub in range(g):
                        mc = m0 + sub
                        for kc in range(Kc):
                            nc.tensor.matmul(
                                p[:, sub * 128:(sub + 1) * 128],
                                w[:, kc, mc * 128:(mc + 1) * 128],
                                rhs_fn(kc),
                                start=(kc == 0), stop=(kc == Kc - 1))
                    if gi == 0 and mid is not None:
                        mid()
                    consume(p, m0, g)

            embT_ = embT
            TILE_NAMES.clear()

            # ---------- precompute EG = embT @ Wg_top ----------
            EG = cpool.tile([128, KC_D, ROWS], F32, tag="EG")

            def eg_consume(p, m0, g):
                nc.vector.tensor_copy(EG[:, m0:m0 + g, :], p[:])
            mm_layer("Wgt", KC_D, KC_D, lambda kc: embTbf[:, kc, :], eg_consume)

            # ---------- Picard sweeps ----------
            CTXs = [cpool.tile([128, KC_D, ROWS], BF16, tag=f"CTX{i}",
                               name=f"CTX{i}") for i in range(2)]
            TFs = [cpool.tile([128, KC_D, ROWS], F32, tag=f"TF{i}",
                              name=f"TF{i}") for i in range(2)]
            Hbf = cpool.tile([128, KC_D, ROWS], BF16, tag="Hbf")

            for it in range(n_iters):
                first = (it == 0)
                last = (it == n_iters - 1)
                CTX = embTbf if first else CTXs[it % 2]
                TFc = None if first else TFs[it % 2]
                CTXn = CTXs[(it + 1) % 2]
                TFn = TFs[(it + 1) % 2]

                # A = gelu(ctx@V0)  (single ScalarE op per group, PSUM -> SBUF)
                Abf = apool.tile([128, KC_H, ROWS], BF16, tag="Abf")

                def a_consume(p, m0, g):
                    nc.scalar.activation(Abf[:, m0:m0 + g, :], p[:], AF.Gelu)
                mm_layer("V0", KC_D, KC_H, lambda kc: CTX[:, kc, :], a_consume)

                # TGT = gelu(A@V1); also accumulate sum(TGT^2) rows
                TGTbf = apool.tile([128, KC_D, ROWS], BF16, tag="TGTbf", bufs=2)
                ssp = sps.tile([1, 2 * ROWS], F32, tag="sums")
                sq = npool.tile([128, KC_D, ROWS], BF16, tag="sqh", bufs=2)

                def t_consume(p, m0, g):
                    nc.scalar.activation(TGTbf[:, m0:m0 + g, :], p[:], AF.Gelu)
                    nc.vector.tensor_tensor(
                        sq[:, m0:m0 + g, :], TGTbf[:, m0:m0 + g, :],
                        TGTbf[:, m0:m0 + g, :], ALU.mult)
                    for sub in range(g):
                        mc = m0 + sub
                        nc.tensor.matmul(ssp[:, 0:ROWS], ones_col_bf[:],
                                         sq[:, mc, :],
                                         start=(mc == 0), stop=(mc == KC_D - 1))
                mm_layer("V1", KC_H, KC_D, lambda kc: Abf[:, kc, :], t_consume)

                # l2n scale s on DVE (runs under W1's first matmul group)
                r_l2 = _nr_rsqrt(nc, npool, ssp[:, 0:ROWS], ROWS, "l2n",
                                 iters=1)
                r_l2bf = npool.tile([1, ROWS], BF16, tag="r_l2bf")
                nc.vector.tensor_copy(r_l2bf[:], r_l2[:])
                rb = npool.tile([128, ROWS], F32, tag="rb")

                def w1_mid():
                    rb_p = pps.tile([128, ROWS], F32, tag="mmps")
                    nc.tensor.matmul(rb_p[:], ones_row_bf[:], r_l2bf[:],
                                     start=True, stop=True)
                    nc.vector.tensor_copy(rb[:], rb_p[:])

                # U = gelu(s*(TGT@W1))
                Ubf = apool.tile([128, KC_D, ROWS], BF16, tag="Ubf", bufs=2)

                def u_consume(p, m0, g):
                    ysc = apool.tile([128, g * 128], F32, tag=f"ysc{m0 % 8}")
                    for sub in range(g):
                        sl = slice(sub * 128, (sub + 1) * 128)
                        nc.vector.tensor_tensor(ysc[:, sl], p[:, sl], rb[:],
                                                ALU.mult)
                        nc.scalar.activation(Ubf[:, m0 + sub, :], ysc[:, sl],
                                             AF.Gelu)
                mm_layer("W1", KC_D, KC_D, lambda kc: TGTbf[:, kc, :], u_consume,
                         mid=w1_mid)

                # G = sigmoid(EG + U@(W2@Wg_bot)) = 0.5 + 0.5*tanh(x/2)
                Gsb = apool.tile([128, KC_D, ROWS], F32, tag="Gsb")

                def g_consume(p, m0, g):
                    gin = apool.tile([128, g * 128], F32, tag=f"sc{m0 % 8}")
                    nc.vector.tensor_tensor(gin[:], p[:], EG[:, m0:m0 + g, :],
                                            ALU.add)
                    th = apool.tile([128, g * 128], F32, tag=f"th{m0 % 8}")
                    nc.scalar.activation(th[:], gin[:], AF.Tanh, scale=0.5)
                    nc.vector.tensor_scalar(Gsb[:, m0:m0 + g, :], th[:], 0.5,
                                            0.5, ALU.mult, ALU.add)
                mm_layer("W2Wg", KC_D, KC_D, lambda kc: Ubf[:, kc, :], g_consume)

                # CF = U@W2; hpre = G*(CF + TFc - EMB) + EMB (TFc pre-alpha'd)
                hpre_bf = npool.tile([128, KC_D, ROWS], BF16, tag="hpre_bf",
                                     bufs=2)
                hsq = npool.tile([128, KC_D, ROWS], BF16, tag="sqh", bufs=2)

                def hp_consume(p, m0, g):
                    t1 = apool.tile([128, g * 128], F32, tag=f"t1_{m0 % 8}")
                    if first:
                        nc.vector.tensor_tensor(
                            t1[:], p[:], embT_[:, m0:m0 + g, :], ALU.subtract)
                    else:
                        nc.vector.tensor_tensor(
                            t1[:], p[:], TFc[:, m0:m0 + g, :], ALU.add)
                        nc.vector.tensor_tensor(
                            t1[:], t1[:], embT_[:, m0:m0 + g, :], ALU.subtract)
                    nc.vector.tensor_tensor(t1[:], t1[:], Gsb[:, m0:m0 + g, :],
                                            ALU.mult)
                    nc.vector.tensor_tensor(hpre_bf[:, m0:m0 + g, :], t1[:],
                                            embT_[:, m0:m0 + g, :], ALU.add)
                    nc.vector.tensor_tensor(hsq[:, m0:m0 + g, :],
                                            hpre_bf[:, m0:m0 + g, :],
                                            hpre_bf[:, m0:m0 + g, :], ALU.mult)
                mm_layer("W2", KC_D, KC_D, lambda kc: Ubf[:, kc, :], hp_consume,
                         group=2)

                # ---- sweep tail: R/RWt on pre-LN hpre + LN stats, all PE
                # work ordered so nothing in the in-order queue waits ----
                s12 = sps.tile([1, 2 * ROWS], F32, tag="sums")

                def stat_mms():
                    for mc in range(KC_D):
                        nc.tensor.matmul(s12[:, 0:ROWS], ones_col_bf[:],
                                         hpre_bf[:, mc, :],
                                         start=(mc == 0), stop=(mc == KC_D - 1))
                    for mc in range(KC_D):
                        nc.tensor.matmul(s12[:, ROWS:2 * ROWS], ones_col_bf[:],
                                         hsq[:, mc, :],
                                         start=(mc == 0), stop=(mc == KC_D - 1))

                def stats_chain():
                    mrow = npool.tile([1, ROWS], F32, tag="mrow")
                    nc.vector.tensor_scalar(mrow[:], s12[:, 0:ROWS], 1.0 / D,
                                            None, ALU.mult)
                    var = npool.tile([1, ROWS], F32, tag="var")
                    nc.vector.tensor_tensor(var[:], mrow[:], mrow[:], ALU.mult)
                    nc.vector.scalar_tensor_tensor(
                        var[:], s12[:, ROWS:2 * ROWS], 1.0 / D, var[:],
                        ALU.mult, ALU.subtract)
                    nc.vector.tensor_scalar(var[:], var[:], 1e-5, None, ALU.add)
                    r_ln = _nr_rsqrt(nc, npool, var[:], ROWS, "ln", iters=1)
                    return mrow, r_ln

                if last:
                    stat_mms()
                    mrow, r_ln = stats_chain()
                    mrow_bf = npool.tile([1, ROWS], BF16, tag="mrow_bf")
                    nc.vector.tensor_copy(mrow_bf[:], mrow[:])
                    r_ln_bf = npool.tile([1, ROWS], BF16, tag="r_ln_bf")
                    nc.vector.tensor_copy(r_ln_bf[:], r_ln[:])
                    mb_p = pps.tile([128, ROWS], F32, tag="mmps")
                    nc.tensor.matmul(mb_p[:], ones_row_bf[:], mrow_bf[:],
                                     start=True, stop=True)
                    rb2_p = pps.tile([128, ROWS], F32, tag="mmps")
                    nc.tensor.matmul(rb2_p[:], ones_row_bf[:], r_ln_bf[:],
                                     start=True, stop=True)
                    for kc in range(KC_D):
                        d_ = npool.tile([128, ROWS], F32, tag=f"lnd{kc % 3}",
                                        name=f"lnd{kc}")
                        nc.vector.tensor_tensor(d_[:], hpre_bf[:, kc, :],
                                                mb_p[:], ALU.subtract)
                        nc.vector.tensor_tensor(Hbf[:, kc, :], d_[:], rb2_p[:],
                                                ALU.mult)
                    continue

                rq_list = []

                def rq_mms(wname, m0, g):
                    p = rqps.tile([128, g * 128], F32, tag="rqp")
                    for sub in range(g):
                        mc = m0 + sub
                        for kc in range(KC_D):
                            nc.tensor.matmul(
                                p[:, sub * 128:(sub + 1) * 128],
                                wsb[wname][:, kc, mc * 128:(mc + 1) * 128],
                                hpre_bf[:, kc, :],
                                start=(kc == 0), stop=(kc == KC_D - 1))
                    rq_list.append((p, m0, g))

                rq_mms("R", 0, 3)
                stat_mms()
                rq_mms("R", 3, 3)
                rq_mms("RWt", 0, 3)
                mrow, r_ln = stats_chain()
                mrow_bf = npool.tile([1, ROWS], BF16, tag="mrow_bf")
                nc.vector.tensor_copy(mrow_bf[:], mrow[:])
                as_row = npool.tile([1, ROWS], BF16, tag="as_row")
                nc.vector.tensor_scalar(as_row[:], r_ln[:], ALPHA, None,
                                        ALU.mult)
                # broadcasts sit in the PE queue behind RWt g0: stats are
                # ready by then, so they never block the stream
                mb_p = pps.tile([128, ROWS], F32, tag="mmps")
                nc.tensor.matmul(mb_p[:], ones_row_bf[:], mrow_bf[:],
                                 start=True, stop=True)
                asb_p = pps.tile([128, ROWS], F32, tag="mmps")
                nc.tensor.matmul(asb_p[:], ones_row_bf[:], as_row[:], start=True,
                                 stop=True)
                mb = npool.tile([128, ROWS], F32, tag="mb")
                nc.vector.tensor_copy(mb[:], mb_p[:])
                asb = npool.tile([128, ROWS], F32, tag="asb")
                nc.vector.tensor_copy(asb[:], asb_p[:])
                rq_mms("RWt", 3, 3)

                # boundary rows 0:4 of CTXn/TFn (lag-2 halo, consumed at the
                # tail so the in-order DVE queue never waits on the AG)
                if it >= 1:
                    blocks = npool.tile([128, 8, KC_D * 8], F32, tag="blocks")
                    nc.sync.dma_start(
                        blocks[:],
                        halo_out[it - 1].ap().rearrange("(r p) f -> p r f",
                                                        p=128))
                    hacc = npool.tile([128, KC_D * 8], F32, tag="hacc")
                    nc.vector.tensor_scalar(hacc[:], blocks[:, 0, :],
                                            sel[:, 0:1], None, ALU.mult)
                    for r in range(1, N_CORES):
                        nc.vector.scalar_tensor_tensor(
                            hacc[:], blocks[:, r, :], sel[:, r:r + 1],
                            hacc[:], ALU.mult, ALU.add)
                    ha = hacc[:].rearrange("p (k c) -> p k c", k=KC_D)
                    for kc in range(KC_D):
                        nc.vector.tensor_tensor(
                            CTXn[:, kc, 0:4], ha[:, kc, 0:4],
                            embT_[:, kc, 0:4], ALU.add)
                    nc.vector.tensor_copy(TFn[:, :, 0:4], ha[:, :, 4:8])
                else:
                    for kc in range(KC_D):
                        nc.vector.tensor_copy(CTXn[:, kc, 0:4],
                                              embT_[:, kc, 0:4])
                    nc.vector.memset(TFn[:, :, 0:4], 0.0)

                # deferred consumes: CTXn/TFn interiors + halo edges
                Ppre = apool.tile([128, KC_D, ROWS], F32, tag="Ppre")
                hQe = apool.tile([128, KC_D, 4], F32, tag="hQe", bufs=2)
                for gi, (p, m0, g) in enumerate(rq_list):
                    is_r = gi < 2
                    for sub in range(g):
                        mc = m0 + sub
                        sl = slice(sub * 128, (sub + 1) * 128)
                        tmp = apool.tile([128, ROWS], F32, tag=f"rtmp{mc % 4}")
                        nc.vector.scalar_tensor_tensor(
                            tmp[:], mb[:], csn[:, mc, 0:1] if is_r
                            else csn[:, mc, 1:2], p[:, sl], ALU.mult, ALU.add)
                        if is_r:
                            nc.vector.tensor_tensor(Ppre[:, mc, :], tmp[:],
                                                    asb[:], ALU.mult)
                            nc.vector.tensor_tensor(
                                CTXn[:, mc, 4:ROWS], Ppre[:, mc, 0:ROWS - 4],
                                embT_[:, mc, 4:ROWS], ALU.add)
                        else:
                            nc.vector.tensor_tensor(
                                TFn[:, mc, 4:ROWS], tmp[:, 0:ROWS - 4],
                                asb[:, 0:ROWS - 4], ALU.mult)
                            nc.vector.tensor_tensor(
                                hQe[:, mc, :], tmp[:, ROWS - 4:ROWS],
                                asb[:, ROWS - 4:ROWS], ALU.mult)

                # launch my halo (P edge + Q edge) for sweep it+2
                if it + 2 < n_iters:
                    hio = halo_in[it].ap().rearrange("p (k c) -> p k c", k=KC_D)
                    nc.sync.dma_start(hio[:, :, 0:4], Ppre[:, :, ROWS - 4:ROWS])
                    nc.sync.dma_start(hio[:, :, 4:8], hQe[:])
                    nc.gpsimd.collective_compute(
                        "AllGather", ALU.bypass, replica_groups=rg,
                        ins=[halo_in[it][:]], outs=[halo_out[it][:]])

            # ---------- final 2-chunk AllGather of Hbf ----------
            nc.sync.dma_start(hfa_in.ap().rearrange("p (k c) -> p k c", k=KC_D),
                              Hbf[:, :, 0:HALF])
            nc.gpsimd.collective_compute(
                "AllGather", ALU.bypass, replica_groups=rg,
                ins=[hfa_in[:]], outs=[hfa_out[:]])
            nc.sync.dma_start(hfb_in.ap().rearrange("p (k c) -> p k c", k=KC_D),
                              Hbf[:, :, HALF:ROWS])
            nc.gpsimd.collective_compute(
                "AllGather", ALU.bypass, replica_groups=rg,
                ins=[hfb_in[:]], outs=[hfb_out[:]])

        # ---------- lm_head: logits^T = Wl^T @ H^T, vocab-sharded, bf16.
        # Wl is streamed (the pool allocates into V0/V1's freed zone, so the
        # stream starts during the final sweep); the H AllGather is split in
        # two row-halves so chunk b overlaps phase-a matmuls. ----------
        with (
            tc.tile_pool(name="lmpool", bufs=16) as lmpool,
            tc.tile_pool(name="opool", bufs=4) as opool,
            tc.tile_pool(name="lps", bufs=6, space="PSUM") as lps,
        ):
            Hra = lmpool.tile([128, KC_D, N_CORES * HALF], BF16, tag="Hra",
                              bufs=1)
            nc.sync.dma_start(
                Hra[:].rearrange("p k (r c) -> p k r c", r=N_CORES),
                hfa_out.ap().rearrange("(r p) (k c) -> p k r c", p=128, k=KC_D))
            Hrb = lmpool.tile([128, KC_D, N_CORES * HALF], BF16, tag="Hrb",
                              bufs=1)
            nc.sync.dma_start(
                Hrb[:].rearrange("p k (r c) -> p k r c", r=N_CORES),
                hfb_out.ap().rearrange("(r p) (k c) -> p k r c", p=128, k=KC_D))

            for half, Hr in ((0, Hra), (1, Hrb)):
                for vc in range(NV):
                    wl_t = lmpool.tile([128, KC_D, 128], BF16, tag="wl")
                    nc.sync.dma_start(wl_t[:], wl_ext[vc])
                    p = lps.tile([128, N_CORES * HALF], F32, tag="lmp")
                    for kc in range(KC_D):
                        nc.tensor.matmul(
                            p[:], wl_t[:, kc, :], Hr[:, kc, :],
                            start=(kc == 0), stop=(kc == KC_D - 1))
                    osb = opool.tile([128, N_CORES * HALF], F32, tag="osb")
                    if vc % 2 == 0:
                        nc.vector.tensor_copy(osb[:], p[:])
                    else:
                        nc.scalar.copy(osb[:], p[:])
                    nc.sync.dma_start(
                        out_ext[half, vc * 128:(vc + 1) * 128, :], osb[:])

    nc.compile()
    return nc


def _get_built(n_iters=N_ITERS):
    if n_iters not in _BUILD_CACHE:
        _BUILD_CACHE[n_iters] = build(n_iters)
    return _BUILD_CACHE[n_iters]


def _prep_in_maps(token_ids, embedding, V0, b0, V1, b1, W1, c1, W2, c2, Wg, bg,
                  Wt, gamma, beta, Wl, R_weight):
    f64 = np.float64
    for z in (b0, b1, c1, c2, bg, beta):
        assert np.count_nonzero(np.asarray(z)) == 0, "nonzero bias unsupported"
    assert np.allclose(np.asarray(gamma), 1.0), "gamma != 1 unsupported"

    tok = np.asarray(token_ids).astype(np.int64)           # [B, T]
    emb = np.asarray(embedding, f64)[tok]                  # [B, T, D]
    emb = emb / np.maximum(np.linalg.norm(emb, axis=-1, keepdims=True), 1e-12)
    rows = emb.transpose(1, 0, 2).reshape(T * B, D)        # row = t*4+b

    bf = ml_dtypes.bfloat16
    R_bf = _t_layout(np.asarray(R_weight, f64)).astype(bf)
    RWt_bf = _t_layout(np.asarray(R_weight, f64) @ np.asarray(Wt, f64)).astype(bf)
    wt = {
        "R": R_bf,
        "V0": _t_layout(np.asarray(V0, f64)).astype(bf),
        "V1": _t_layout(np.asarray(V1, f64)).astype(bf),
        "W1": _t_layout(np.asarray(W1, f64)).astype(bf),
        "W2": _t_layout(np.asarray(W2, f64)).astype(bf),
        "RWt": RWt_bf,
        "Wgt": _t_layout(np.asarray(Wg, f64)[:D]).astype(bf),
        "W2Wg": _t_layout(np.asarray(W2, f64) @ np.asarray(Wg, f64)[D:]).astype(bf),
    }
    # negated col-sums of the bf16-rounded R / RWt, in T-layout [128, KC_D]
    csn = np.zeros((128, KC_D, 2), np.float32)
    for i, w_bf in enumerate((R_bf, RWt_bf)):
        cs = -w_bf.astype(f64).sum(axis=(0, 1))            # [-colsum] over K
        csn[:, :, i] = cs.reshape(KC_D, 128).T
    wl_bf = np.asarray(Wl, f64).astype(bf)

    in_maps = []
    for c in range(N_CORES):
        block = rows[c * ROWS:(c + 1) * ROWS].T            # [D, 128]
        embT = np.ascontiguousarray(
            block.reshape(KC_D, 128, ROWS).transpose(1, 0, 2)).astype(np.float32)
        sel = np.zeros((128, 8), np.float32)
        if c > 0:
            sel[:, c - 1] = 1.0
        wl_shard_cols = np.zeros((D, VPAD), bf)
        lo = c * VSHARD
        hi = min(V, lo + VSHARD)
        wl_shard_cols[:, :hi - lo] = wl_bf[:, lo:hi]
        wl_shard = _t_layout(wl_shard_cols)                 # [128, KC_D, VPAD]
        wl_shard = np.ascontiguousarray(
            wl_shard.reshape(128, KC_D, NV, 128).transpose(2, 0, 1, 3))
        m = {"embT": embT, "sel": sel, "csn": csn, "wl": wl_shard}
        for name, w in wt.items():
            m[f"wb_{name}"] = w
        in_maps.append(m)
    return in_maps


def kernel(**inputs):
    global LAST_RESULT
    in_maps = _prep_in_maps(**{k: np.asarray(v) for k, v in inputs.items()})
    nc = _get_built()
    trace = bool(os.environ.get("KERNEL_TRACE"))
    res = run_bass_kernel_spmd(nc, in_maps, core_ids=list(range(N_CORES)),
                               trace=trace)
    LAST_RESULT = res
    parts = []
    for c in range(N_CORES):
        o = res.results[c]["out"]                          # [2, VPAD, 8*HALF]
        o = o.reshape(2, VPAD, N_CORES, HALF)
        o = o.transpose(1, 2, 0, 3).reshape(VPAD, T * B)   # row = core*128+half*64+i
        parts.append(o[:VSHARD])
    L = np.concatenate(parts, axis=0)[:V]                  # [V, T*B]
    out = np.ascontiguousarray(
        L.reshape(V, T, B).transpose(2, 1, 0)).astype(np.float32)
    return out


if __name__ == "__main__":
    pass
